# revision 33
# baseline (speedup 1.0000x reference)
"""Trainium2 Bass kernel for nn_NeuralNetwork_86990267613505 (topk_masking).

Network (per reference):
  cx = sigmoid(tanh(input @ W_c1.T + b_c1) @ W_c2.T)          # [B] gate
  x  = kwta(input @ W1.T + b1, k=int(cx*1024))                # [B,1024]
  x  = kwta(x @ W2.T + b2,     k=int(cx*512))                 # [B,512]
  x  = kwta(x @ W3.T + b3,     k=int(cx*1024))                # [B,1024]
  out = x @ W4.T                                              # [B,1024]

Sharding: the two big matmuls (contraction over S2=32768) are column-sharded
over the contraction dim across 8 cores (4096 each); partial sums are combined
with a single fused ReduceScatter of [B, 512+1024] which also distributes the
batch (32 rows per core).  Everything after is data-parallel per core.

kwta: per-row exact k-th-largest via 7-pass radix-5 bisection (probes
replicated 4x across partitions; the per-pass replica-sum of count>=k runs
as a PE matmul against a width-scaled [p==q mod 32] matrix, which sums and
broadcasts in one instruction), then band extraction (bf16 compares, f32
values) + one max8 + indicator-pick, then mask = (x >= thresh) * x.

Precision strategy (the kWTA selection cliff needs ~1e-5 absolute accuracy
on every pre-kwta activation): the streamed matmuls use a 3-pass bf16 hi/lo
decomposition (hi*hi + hi*lo + lo*hi), biases are folded in as a K=2 matmul
of hi/lo bias rows, and the tail matmuls stay fp32 except the final output
matmul which runs float32r (its ~2^-12 noise does not pass any selection).
"""

import numpy as np

import concourse.bacc as bacc
import concourse.mybir as mybir
import concourse.tile as tile
from concourse import bass_utils

F32 = mybir.dt.float32
F32R = mybir.dt.float32r
BF16 = mybir.dt.bfloat16
I32 = mybir.dt.int32
I16 = mybir.dt.int16
ALU = mybir.AluOpType
ACTF = mybir.ActivationFunctionType

HID = 512
N1 = 2 * HID      # 1024
N3 = 1024         # HEADS
R = 32            # rows per core after scatter
C = 4             # partition replication for probing
BIG = 1e30
N_PASS = 7        # radix-5 bisection passes (top-8 band needs >=7)


class Cfg:
    def __init__(self, S2=32768, B=256, NC=8, chunk=4, debug=False):
        assert B // NC == R
        self.S2, self.B, self.NC, self.chunk = S2, B, NC, chunk
        self.debug = debug
        self.no_collective = False
        self.loop_n = 0
        self.phase = None  # None | 'notail'
        self.pe_reduce = True   # PE-matmul replica reduction in kwta
        self.warmup = False     # PE pre-warm (helps one-shot only)
        self.KSH = S2 // NC            # contraction shard per core
        self.KT = self.KSH // 128      # k-tiles
        assert self.KT % chunk == 0
        self.SW = B + 3 * HID          # stream free width per k-tile
        # b-tiles: chunks of <=128 rows of the full batch
        self.b_tiles = [(s, min(128, B - s)) for s in range(0, B, 128)]


def _floorize3(nc, sb, val_ap, rows, name):
    """floor(val) per column for val >= 0 on [rows,3], HW float->int is RNE."""
    ki = sb.tile([rows, 3], I32, name=f"{name}_i")
    kb = sb.tile([rows, 3], F32, name=f"{name}_b")
    cmp = sb.tile([rows, 3], F32, name=f"{name}_c")
    kf = sb.tile([rows, 3], F32, name=f"{name}_f")
    nc.vector.tensor_copy(ki[:], val_ap)
    nc.vector.tensor_copy(kb[:], ki[:])
    nc.vector.tensor_tensor(cmp[:], kb[:], val_ap, ALU.is_gt)
    nc.vector.tensor_sub(kf[:], kb[:], cmp[:])
    return kf


def _pe_keepalive(nc, ps, src_ap, lname, i):
    pdum = ps.tile([1, 1], F32, tag="tp", bufs=2, name=f"{lname}_pd{i}")
    nc.tensor.matmul(pdum[:], src_ap, src_ap, start=True, stop=True)


def _kwta(nc, sb, ps, x_ap, ktile, kcol, n, consts, lname, pe_reduce=True):
    """x_ap: [128, n] fp32 SBUF (rows replicated 4x: partition 32c+r = row r).
    ktile[:, kcol]: [128,1] fp32 float(k).  Returns masked [R, n] f32 tile.

    Radix-5 bisection; the per-pass replica-sum of the count>=k indicator is
    done on the PE (matmul with a [p==q mod 32] 0/1 matrix), which both sums
    over the 4 replicas and broadcasts the result to all 128 partitions in a
    single instruction, replacing a 6-op DVE shuffle chain.
    """
    iota16 = consts["iota16"]
    fpw, rmatw = consts["fpw"], consts["rmatw"]

    # lp[:,0] = lo, lp[:,1] = probe.  Init const: lo=-16, probe = frac*32-16.
    lp = consts["lp0"]

    xb = sb.tile([128, n], BF16, tag="kw_xb", name=f"{lname}_xb0")
    nc.vector.tensor_copy(xb[:], x_ap)
    trash = sb.tile([128, n], BF16, tag="kw_tr", name=f"{lname}_tr0")
    cnt = sb.tile([128, 1], F32, tag="kw_cnt", bufs=2, name=f"{lname}_cnt0")
    bandv = sb.tile([R, n], F32, tag="kw_bv", name=f"{lname}_bv0")
    nc.vector.memset(bandv[:], -BIG)
    for p in range(N_PASS):
        # probes/lo update fused: lp_new = fpw[p] + w5*j + lo, with the w5
        # scaling folded into the rmatw[p] matmul weights
        nc.vector.tensor_scalar(
            trash[:], xb[:], lp[:, 1:2], None, ALU.is_ge, ALU.add,
            accum_out=cnt[:],
        )
        ge = sb.tile([128, 1], F32, tag="kw_ge", bufs=2, name=f"{lname}_ge{p}")
        nc.vector.tensor_scalar(ge[:], cnt[:], ktile[:, kcol:kcol + 1], None, ALU.is_ge)
        if pe_reduce:
            jp = ps.tile([128, 1], F32, tag="tp", bufs=2, name=f"{lname}_jp{p}")
            nc.tensor.matmul(jp[:], rmatw[:, 128 * p:128 * (p + 1)], ge[:],
                             start=True, stop=True)
            j_ap = jp[:, 0:1]
        else:
            sh64 = sb.tile([64, 1], F32, tag="kw_s64", bufs=2, name=f"{lname}_s64_{p}")
            f2 = sb.tile([64, 1], F32, tag="kw_f2", bufs=2, name=f"{lname}_f2_{p}")
            sh32 = sb.tile([32, 1], F32, tag="kw_s32", bufs=2, name=f"{lname}_s32_{p}")
            jall = sb.tile([128, 1], F32, tag="kw_j", bufs=2, name=f"{lname}_j{p}")
            nc.vector.tensor_copy(sh64[:], ge[64:128, :])
            nc.vector.tensor_add(f2[:], ge[0:64, :], sh64[:])
            nc.vector.tensor_copy(sh32[:], f2[32:64, :])
            nc.vector.tensor_add(jall[0:32, :], f2[0:32, :], sh32[:])
            nc.vector.tensor_copy(jall[32:64, :], jall[0:32, :])
            nc.vector.tensor_copy(jall[64:128, :], jall[0:64, :])
            jsc = sb.tile([128, 1], F32, tag="kw_jsc", bufs=2, name=f"{lname}_jsc{p}")
            nc.vector.tensor_scalar(jsc[:], jall[:], 32.0 * 0.2 ** (p + 1),
                                    None, ALU.mult)
            j_ap = jsc[:, 0:1]
            _pe_keepalive(nc, ps, ge[0:1, 0:1], lname, p)
        lp_new = sb.tile([128, 2], F32, tag="kw_lp", bufs=2, name=f"{lname}_lp{p+1}")
        nc.vector.tensor_scalar(lp_new[:], fpw[:, 2 * p:2 * (p + 1)],
                                j_ap, lp[:, 0:1], ALU.add, ALU.add)
        lp = lp_new

    lo = lp[:, 0:1]
    w_fin = 32.0 * 0.2 ** N_PASS
    hi = sb.tile([128, 1], F32, name=f"{lname}_hi")
    nc.vector.tensor_scalar(hi[:], lo, w_fin, None, ALU.add)
    # band membership from bf16 compares (monotone rounding keeps the true
    # k-th inside), band VALUES from exact f32 x.
    x_r = x_ap[0:R, :]
    bhi = sb.tile([R, n], BF16, tag="kw_bhi", name=f"{lname}_bhi0")
    binb = sb.tile([R, n], I16, tag="kw_binb", name=f"{lname}_binb0")
    nc.vector.tensor_scalar(bhi[:], xb[0:R, :], hi[0:R, 0:1], None, ALU.is_lt)
    nc.vector.scalar_tensor_tensor(
        binb[:], xb[0:R, :], lo[0:R, 0:1], bhi[:], ALU.is_ge, ALU.mult)
    # c_hi = count(x >= hi); issued after the band compares so the
    # band->max8 critical path advances first (chi is only needed at pick)
    chi = sb.tile([128, 1], F32, name=f"{lname}_chi")
    nc.vector.tensor_scalar(
        trash[:], xb[:], hi[:, 0:1], None, ALU.is_ge, ALU.add, accum_out=chi[:],
    )
    _pe_keepalive(nc, ps, hi[0:1, 0:1], lname, "ka")
    nc.vector.copy_predicated(bandv[:], binb[:], x_r)
    # top-8 of band (band width 0.002 keeps expected band size ~1.5)
    m8 = sb.tile([R, 8], F32, name=f"{lname}_m8")
    nc.vector.max(m8[:], bandv[:])
    # pick (k - c_hi - 1)-th
    rf = sb.tile([R, 1], F32, name=f"{lname}_rf")
    nc.vector.tensor_sub(rf[:], ktile[0:R, kcol:kcol + 1], chi[0:R, :])
    nc.vector.tensor_scalar(rf[:], rf[:], 1.0, None, ALU.subtract)
    nc.vector.tensor_scalar(rf[:], rf[:], 0.0, 7.0, ALU.max, ALU.min)
    ind = sb.tile([R, 8], F32, name=f"{lname}_ind")
    nc.vector.tensor_scalar(ind[:], iota16[0:R, 0:8], rf[:, 0:1], None, ALU.is_equal)
    iv = sb.tile([R, 8], F32, name=f"{lname}_iv")
    nc.vector.tensor_mul(iv[:], ind[:], m8[:])
    vk = sb.tile([R, 1], F32, name=f"{lname}_vk")
    nc.vector.reduce_sum(vk[:], iv[:], axis=mybir.AxisListType.X)
    # thresh = k>=1 ? vk : +BIG
    g = sb.tile([R, 1], F32, name=f"{lname}_g")
    ga = sb.tile([R, 1], F32, name=f"{lname}_ga")
    gb = sb.tile([R, 1], F32, name=f"{lname}_gb")
    thr = sb.tile([R, 1], F32, name=f"{lname}_thr")
    nc.vector.tensor_scalar(g[:], ktile[0:R, kcol:kcol + 1], 1.0, None, ALU.is_ge)
    nc.vector.tensor_scalar(ga[:], g[:], -BIG, BIG, ALU.mult, ALU.add)
    nc.vector.tensor_scalar(gb[:], g[:], vk[:, 0:1], None, ALU.mult)
    nc.vector.tensor_add(thr[:], ga[:], gb[:])
    # masked = (x >= thresh) * x
    masked = sb.tile([R, n], F32, tag="kw_mask", name=f"{lname}_masked")
    nc.vector.scalar_tensor_tensor(
        masked[:], x_r, thr[:, 0:1], x_r, ALU.is_ge, ALU.mult)
    return masked, thr


def _transpose_chunks(nc, sb, pst, masked, n, ident, rep, lname, dtype=F32):
    """masked [R, n] f32 -> list of xT tiles: [128, C*R] (rep) or [128, R]."""
    tiles = []
    for ch in range(n // 128):
        pt = pst.tile([128, R], F32, tag="tp", name=f"{lname}_pt{ch}")
        nc.tensor.transpose(pt[:], masked[:, 128 * ch:128 * (ch + 1)],
                            ident[0:R, 0:R])
        if rep:
            xt = sb.tile([128, C * R], dtype, tag="kw_xt", bufs=8,
                         name=f"{lname}_xt{ch}")
            nc.vector.tensor_copy(
                xt[:].rearrange("p (c r) -> p c r", c=C),
                pt[:, :].unsqueeze(1).broadcast_to([128, C, R]),
            )
        else:
            xt = sb.tile([128, R], dtype, tag="kw_xt", bufs=8,
                         name=f"{lname}_xt{ch}")
            nc.vector.tensor_copy(xt[:], pt[:])
        tiles.append(xt)
    return tiles


def build_nc(cfg: Cfg):
    nc = bacc.Bacc("TRN2", target_bir_lowering=False, debug=False,
                   num_devices=cfg.NC)
    B, NC, KT, SW, chunk = cfg.B, cfg.NC, cfg.KT, cfg.SW, cfg.chunk

    stream_d = nc.dram_tensor("stream", [KT, 128, 2, SW], BF16, kind="ExternalInput")
    ident_d = nc.dram_tensor("ident", [128, 128], F32, kind="ExternalInput")
    brow_d = nc.dram_tensor("brow", [2, 3 * HID], BF16, kind="ExternalInput")
    b2rep_d = nc.dram_tensor("b2rep", [128, HID], F32, kind="ExternalInput")
    b3rep_d = nc.dram_tensor("b3rep", [128, N3], F32, kind="ExternalInput")
    wc2rep_d = nc.dram_tensor("wc2rep", [128, HID], F32, kind="ExternalInput")
    fracpair_d = nc.dram_tensor("fracpair", [128, 2], F32, kind="ExternalInput")
    lp0_d = nc.dram_tensor("lp0", [128, 2], F32, kind="ExternalInput")
    fpw_d = nc.dram_tensor("fpw", [N_PASS, 128, 2], F32, kind="ExternalInput")
    rmat_d = nc.dram_tensor("rmat", [128, 128], F32, kind="ExternalInput")
    rmatw_d = nc.dram_tensor("rmatw", [N_PASS, 128, 128], F32, kind="ExternalInput")
    kmul_d = nc.dram_tensor("kmul", [R, 3], F32, kind="ExternalInput")
    iota16_d = nc.dram_tensor("iota16", [R, 16], F32, kind="ExternalInput")
    w2t_d = nc.dram_tensor("w2t", [N1, HID], F32, kind="ExternalInput")
    w3t_d = nc.dram_tensor("w3t", [HID, N3], F32R, kind="ExternalInput")
    w4t_d = nc.dram_tensor("w4t", [N3, N3], F32R, kind="ExternalInput")
    out_d = nc.dram_tensor("out", [R, N3], F32, kind="ExternalOutput")
    if cfg.debug:
        dbg_rs_d = nc.dram_tensor("dbg_rs", [R, 3 * HID], F32, kind="ExternalOutput")
        dbg_gate_d = nc.dram_tensor("dbg_gate", [R, 8], F32, kind="ExternalOutput")
        dbg_x2_d = nc.dram_tensor("dbg_x2", [R, HID], F32, kind="ExternalOutput")
        dbg_x3_d = nc.dram_tensor("dbg_x3", [R, N3], F32, kind="ExternalOutput")

    import contextlib
    with tile.TileContext(nc) as tc:
        loop_ctx = tc.For_i(0, cfg.loop_n, 1) if cfg.loop_n else contextlib.nullcontext()
        with (
            loop_ctx,
            tc.tile_pool(name="consts", bufs=1) as cp,
            tc.tile_pool(name="stream", bufs=2) as sp,
            tc.tile_pool(name="acc", bufs=1, space="PSUM") as ap,
            tc.tile_pool(name="sb", bufs=1) as sb,
            tc.tile_pool(name="pst", bufs=2, space="PSUM") as pst,
            tc.tile_pool(name="dram", bufs=1, space="DRAM") as dram,
        ):
            # ---- constants ----
            ident = cp.tile([128, 128], F32, name="ident")
            brow = cp.tile([2, 3 * HID], BF16, name="brow")
            ones2 = cp.tile([2, 128], BF16, name="ones2")
            b2rep = cp.tile([128, HID], F32, name="b2rep")
            b3rep = cp.tile([128, N3], F32, name="b3rep")
            wc2rep = cp.tile([128, HID], F32, name="wc2rep")
            fracpair = cp.tile([128, 2], F32, name="fracpair")
            lp0 = cp.tile([128, 2], F32, name="lp0")
            fpw = cp.tile([128, N_PASS * 2], F32, name="fpw")
            rmat = cp.tile([128, 128], F32, name="rmat")
            rmatw = cp.tile([128, N_PASS * 128], F32, name="rmatw")
            kmul = cp.tile([R, 3], F32, name="kmul")
            iota16 = cp.tile([R, 16], F32, name="iota16")
            nc.sync.dma_start(ident[:], ident_d.ap())
            nc.sync.dma_start(brow[:], brow_d.ap())
            nc.vector.memset(ones2[:], 1.0)
            nc.sync.dma_start(b2rep[:], b2rep_d.ap())
            nc.sync.dma_start(b3rep[:], b3rep_d.ap())
            nc.sync.dma_start(wc2rep[:], wc2rep_d.ap())
            nc.sync.dma_start(fracpair[:], fracpair_d.ap())
            nc.sync.dma_start(lp0[:], lp0_d.ap())
            nc.sync.dma_start(fpw[:].rearrange("p (n t) -> p n t", n=N_PASS),
                              fpw_d.ap().transpose([1, 0, 2]))
            nc.sync.dma_start(rmatw[:].rearrange("p (n q) -> p n q", n=N_PASS),
                              rmatw_d.ap().transpose([1, 0, 2]))
            nc.sync.dma_start(rmat[:], rmat_d.ap())
            nc.sync.dma_start(kmul[:], kmul_d.ap())
            nc.sync.dma_start(iota16[:], iota16_d.ap())
            consts = {"ident": ident, "fracpair": fracpair, "rmat": rmat,
                      "lp0": lp0, "fpw": fpw, "rmatw": rmatw,
                      "iota16": iota16}

            # ---- phase A: streamed big matmuls ----
            # one wide accumulator [bsz, 3*HID] per b_tile: each (pass, btile)
            # is a single matmul, sharing one stationary-weight load
            pacc = {}
            for bi, (bs, bsz) in enumerate(cfg.b_tiles):
                pacc[bi] = ap.tile([bsz, 3 * HID], F32, tag="acc", bufs=2,
                                   name=f"pacc{bi}")

            # PE p-state pre-warm: dummy matmuls on ident while the first
            # stream chunk is still in flight, so real matmuls start at the
            # max clock instead of spending their first ~3us ramping.
            if cfg.warmup:
                for wi in range(16):
                    warm = pst.tile([128, 128], F32, tag="tp", bufs=2,
                                    name=f"warm{wi}")
                    nc.tensor.matmul(warm[:], ident[:], ident[:, 0:128],
                                     start=True, stop=True)

            n_chunks = KT // chunk
            for cki in range(n_chunks):
                st = sp.tile([128, chunk * 2 * SW], BF16, tag="st", name=f"st{cki}")
                src = stream_d.ap()[chunk * cki: chunk * (cki + 1)]
                nc.sync.dma_start(
                    st[:].rearrange("p (c t w) -> p c t w", c=chunk, t=2),
                    src.transpose([1, 0, 2, 3]),
                )
                for ki in range(chunk):
                    kt = chunk * cki + ki
                    hi = st[:, (2 * ki) * SW:(2 * ki + 1) * SW]
                    lo = st[:, (2 * ki + 1) * SW:(2 * ki + 2) * SW]
                    first = kt == 0
                    for pi, (xa, wb) in enumerate(((hi, hi), (hi, lo), (lo, hi))):
                        f = first and pi == 0
                        for bi, (bs, bsz) in enumerate(cfg.b_tiles):
                            lhsT = xa[:, bs:bs + bsz]
                            for o in range(3):
                                nc.tensor.matmul(
                                    pacc[bi][:, HID * o:HID * (o + 1)], lhsT,
                                    wb[:, B + HID * o:B + HID * (o + 1)],
                                    start=f, stop=False)
            # bias add via a K=2 matmul of ones against the hi/lo bias rows
            # (closes each accumulation group)
            for bi, (bs, bsz) in enumerate(cfg.b_tiles):
                for o in range(3):
                    nc.tensor.matmul(pacc[bi][:, HID * o:HID * (o + 1)],
                                     ones2[:, 0:bsz],
                                     brow[:, HID * o:HID * (o + 1)],
                                     start=False, stop=True)

            # ---- tail weights (after the stream so they don't delay it) ----
            w2sb = cp.tile([128, 8 * HID], F32, name="w2sb")
            w3sb = cp.tile([128, 4 * N3], F32R, name="w3sb")
            w4sb = cp.tile([128, 8 * N3], F32R, name="w4sb")
            nc.sync.dma_start(
                w2sb[:].rearrange("p (c w) -> p c w", c=8),
                w2t_d.ap().rearrange("(c p) w -> p c w", p=128))
            nc.sync.dma_start(
                w3sb[:].rearrange("p (c w) -> p c w", c=4),
                w3t_d.ap().rearrange("(c p) w -> p c w", p=128))
            nc.sync.dma_start(
                w4sb[:].rearrange("p (c w) -> p c w", c=8),
                w4t_d.ap().rearrange("(c p) w -> p c w", p=128))

            # ---- phase B: stage PSUM->SBUF (bias already added in-PE), RS ----
            rs_in = dram.tile([B, 3 * HID], F32, name="rs_in")
            rs_out = dram.tile([R, 3 * HID], F32, name="rs_out")
            for bi, (bs, bsz) in enumerate(cfg.b_tiles):
                so = sb.tile([bsz, 3 * HID], F32, tag="rsin_sb", bufs=2,
                             name=f"so{bi}")
                if bi == 0:
                    nc.vector.tensor_copy(so[:], pacc[bi][:])
                else:
                    nc.scalar.activation(so[:], pacc[bi][:], ACTF.Identity)
                nc.sync.dma_start(rs_in[bs:bs + bsz, :], so[:])
            if cfg.no_collective:
                nc.sync.dma_start(rs_out[:], rs_in[0:R, :])
            else:
                nc.gpsimd.collective_compute(
                    "ReduceScatter", ALU.add,
                    replica_groups=[list(range(NC))],
                    ins=[rs_in.opt()], outs=[rs_out.opt()],
                )

            # ---- phase C: replicated load + gate ----
            # replica 0 carries the full row (gate cols + x1); replicas 1-3
            # only need the x1 columns for the kwta probe replication.
            xall = sb.tile([128, 3 * HID], F32, name="xall")
            nc.sync.dma_start(xall[0:R, :], rs_out[:])
            for c in range(1, C):
                nc.sync.dma_start(xall[c * R:(c + 1) * R, HID:3 * HID],
                                  rs_out[:, HID:3 * HID])
            th = sb.tile([R, HID], F32, name="tanh")
            nc.scalar.activation(th[:], xall[0:R, 0:HID], ACTF.Tanh)
            ztr = sb.tile([R, HID], F32, name="ztr")
            z = sb.tile([R, 1], F32, name="z")
            nc.vector.scalar_tensor_tensor(
                ztr[:], th[:], 1.0, wc2rep[0:R, :], ALU.mult, ALU.mult,
                accum_out=z[:])
            ez = sb.tile([R, 1], F32, name="ez")
            nc.scalar.activation(ez[:], z[:], ACTF.Exp, scale=-1.0)
            ez1 = sb.tile([R, 1], F32, name="ez1")
            nc.vector.tensor_scalar(ez1[:], ez[:], 1.0, None, ALU.add)
            cx = sb.tile([R, 1], F32, name="cx")
            nc.vector.reciprocal(cx[:], ez1[:])
            kk = sb.tile([R, 3], F32, name="kk")
            nc.vector.tensor_scalar(kk[:], kmul[:], cx[:, 0:1], None, ALU.mult)
            kf3 = _floorize3(nc, sb, kk[:], R, "kf")
            # replicate the three k's to all 128 partitions via the PE
            krep_ps = pst.tile([128, 3], F32, tag="tp", bufs=2, name="krep_ps")
            nc.tensor.matmul(krep_ps[:], rmat[0:R, :], kf3[:], start=True, stop=True)
            krep = sb.tile([128, 3], F32, name="krep")
            nc.vector.tensor_copy(krep[:], krep_ps[:])
            
            do_tail = cfg.phase != "notail"
            if not do_tail:
                nt = sb.tile([R, N3], F32, name="nt")
                nc.vector.tensor_copy(nt[:], xall[0:R, HID:3 * HID])
                nc.vector.tensor_add(nt[:, 0:1], w2sb[0:R, 0:1], w2sb[0:R, 2:3])
                nc.vector.tensor_add(nt[:, 1:2], w2sb[0:R, 1:2], cx[0:R, :])
                nc.vector.tensor_add(nt[:, 2:3], krep[0:R, 0:1], krep[0:R, 1:2])
                nc.vector.tensor_add(nt[:, 3:4], krep[0:R, 2:3], fracpair[0:R, 1:2])
                nc.vector.tensor_add(nt[:, 4:5], ident[0:R, 0:1], iota16[0:R, 0:1])
                nc.vector.tensor_add(nt[:, 5:6], b2rep[0:R, 0:1], b3rep[0:R, 0:1])
                nc.sync.dma_start(out_d.ap(), nt[:])

            if do_tail:
                # ---- layer 1 kwta + mm2 ----
                x1 = xall[:, HID:3 * HID]
                masked1, thr1 = _kwta(nc, sb, pst, x1, krep, 0, N1, consts, "L1", cfg.pe_reduce)
                xt1 = _transpose_chunks(nc, sb, pst, masked1, N1, ident, True, "L1")
                px2 = pacc[0][:, 0:HID]
                w2v = w2sb[:].rearrange("p (c w) -> p c w", c=8)
                for ch in range(8):
                    nc.tensor.matmul(px2, xt1[ch][:], w2v[:, ch, :],
                                     start=(ch == 0), stop=(ch == 7))
                x2 = sb.tile([128, HID], F32, name="x2")
                nc.vector.tensor_add(x2[:], px2, b2rep[:])

                # ---- layer 2 kwta + mm3 ----
                masked2, thr2 = _kwta(nc, sb, pst, x2[:], krep, 1, HID, consts, "L2", cfg.pe_reduce)
                xt2 = _transpose_chunks(nc, sb, pst, masked2, HID, ident, True, "L2",
                                        dtype=F32R)
                px3 = pacc[0][:, HID:HID + N3]
                w3v = w3sb[:].rearrange("p (c w) -> p c w", c=4)
                for ch in range(4):
                    for o in range(2):
                        nc.tensor.matmul(
                            pacc[0][:, HID + 512 * o:HID + 512 * (o + 1)],
                            xt2[ch][:], w3v[:, ch, 512 * o:512 * (o + 1)],
                            start=(ch == 0), stop=(ch == 3))
                x3 = sb.tile([128, N3], F32, name="x3")
                nc.vector.tensor_add(x3[:], px3, b3rep[:])

                # ---- layer 3 kwta + mm4 ----
                masked3, thr3 = _kwta(nc, sb, pst, x3[:], krep, 2, N3, consts, "L3", cfg.pe_reduce)
                xt3 = _transpose_chunks(nc, sb, pst, masked3, N3, ident, False, "L3", dtype=F32R)
                px4 = pacc[1][0:R, 0:N3]
                w4v = w4sb[:].rearrange("p (c w) -> p c w", c=8)
                for ch in range(8):
                    for o in range(2):
                        nc.tensor.matmul(
                            pacc[1][0:R, 512 * o:512 * (o + 1)], xt3[ch][:],
                            w4v[:, ch, 512 * o:512 * (o + 1)],
                            start=(ch == 0), stop=(ch == 7))
                outsb = sb.tile([R, N3], F32, name="outsb")
                nc.vector.tensor_copy(outsb[:, 0:512], pacc[1][0:R, 0:512])
                nc.scalar.activation(outsb[:, 512:1024], pacc[1][0:R, 512:1024],
                                     ACTF.Identity)
                nc.sync.dma_start(out_d.ap(), outsb[:])
                if cfg.debug:
                    nc.sync.dma_start(dbg_rs_d.ap(), xall[0:R, :])
                    gsb = sb.tile([R, 8], F32, name="gsb")
                    nc.vector.tensor_copy(gsb[:, 0:1], cx[:])
                    nc.vector.tensor_copy(gsb[:, 1:2], krep[0:R, 0:1])
                    nc.vector.tensor_copy(gsb[:, 2:3], krep[0:R, 1:2])
                    nc.vector.tensor_copy(gsb[:, 3:4], krep[0:R, 2:3])
                    nc.vector.tensor_copy(gsb[:, 4:5], thr1[:])
                    nc.vector.tensor_copy(gsb[:, 5:6], thr2[:])
                    nc.vector.tensor_copy(gsb[:, 6:7], thr3[:])
                    nc.vector.tensor_copy(gsb[:, 7:8], z[:])
                    nc.sync.dma_start(dbg_gate_d.ap(), gsb[:])
                    nc.sync.dma_start(dbg_x2_d.ap(), x2[0:R, :])
                    nc.sync.dma_start(dbg_x3_d.ap(), x3[0:R, :])

    nc.compile()
    return nc


def host_prepare(inputs, cfg: Cfg):
    """Build per-core in_maps from the full inputs."""
    B, NC, KT, SW, KSH = cfg.B, cfg.NC, cfg.KT, cfg.SW, cfg.KSH
    f32 = np.float32
    inp = np.asarray(inputs["input"], f32)
    W_c1 = np.asarray(inputs["W_c1"], f32)
    b_c1 = np.asarray(inputs["b_c1"], f32)
    W_c2 = np.asarray(inputs["W_c2"], f32)
    W1 = np.asarray(inputs["W1"], f32)
    b1 = np.asarray(inputs["b1"], f32)
    W2 = np.asarray(inputs["W2"], f32)
    b2 = np.asarray(inputs["b2"], f32)
    W3 = np.asarray(inputs["W3"], f32)
    b3 = np.asarray(inputs["b3"], f32)
    W4 = np.asarray(inputs["W4"], f32)

    xT = np.ascontiguousarray(inp.T)          # [S2, B]
    wc1T = np.ascontiguousarray(W_c1.T)       # [S2, HID]
    w1T = np.ascontiguousarray(W1.T)          # [S2, N1]

    import ml_dtypes as _mld
    bcat = np.concatenate([b_c1, b1]).astype(f32) / NC
    bhi = bcat.astype(_mld.bfloat16)
    blo = (bcat - bhi.astype(f32)).astype(_mld.bfloat16)
    frac_col = ((np.arange(128, dtype=f32) // R + 1.0) / 5.0).astype(f32)
    pp = np.arange(128)
    consts = {
        "ident": np.eye(128, dtype=f32),
        "brow": np.stack([bhi, blo], axis=0),
        "b2rep": np.broadcast_to(b2, (128, HID)).copy(),
        "b3rep": np.broadcast_to(b3, (128, N3)).copy(),
        "wc2rep": np.broadcast_to(W_c2[0], (128, HID)).copy(),
        "fracpair": np.stack([np.zeros(128, f32), frac_col], axis=1).copy(),
        "lp0": np.stack([np.full(128, -16.0, f32),
                         (frac_col * 32.0 - 16.0).astype(f32)], axis=1).copy(),
        "fpw": np.stack([np.stack([np.zeros(128, f32),
                                   (frac_col * (32.0 * 0.2 ** (p + 1))).astype(f32)],
                                  axis=1)
                         for p in range(7)], axis=0).copy(),
        "rmat": (pp[:, None] % R == pp[None, :] % R).astype(f32),
        "rmatw": np.stack([(pp[:, None] % R == pp[None, :] % R).astype(f32)
                           * np.float32(32.0 * 0.2 ** (p + 1))
                           for p in range(7)], axis=0).copy(),
        "kmul": np.broadcast_to(
            np.array([2 * HID, HID, N3], f32), (R, 3)).copy(),
        "iota16": np.broadcast_to(np.arange(16, dtype=f32), (R, 16)).copy(),
        "w2t": np.ascontiguousarray(W2.T),
        "w3t": np.ascontiguousarray(W3.T),
        "w4t": np.ascontiguousarray(W4.T),
    }

    import ml_dtypes
    bf16 = ml_dtypes.bfloat16
    in_maps = []
    for c in range(NC):
        sl = slice(c * KSH, (c + 1) * KSH)
        stream = np.concatenate([xT[sl], wc1T[sl], w1T[sl]], axis=1)  # [KSH, SW]
        hi = stream.astype(bf16)
        lo = (stream - hi.astype(f32)).astype(bf16)
        shl = np.stack([hi, lo], axis=1).reshape(KSH, 2, cfg.SW)  # [KSH,2,SW]
        shl = np.ascontiguousarray(shl.reshape(KT, 128, 2, cfg.SW))
        in_maps.append({"stream": shl, **consts})
    return in_maps


_CACHE = {}


def kernel(**inputs) -> np.ndarray:
    cfg = Cfg(S2=inputs["input"].shape[1], B=inputs["input"].shape[0])
    key = (cfg.S2, cfg.B, cfg.NC)
    if key not in _CACHE:
        _CACHE[key] = build_nc(cfg)
    nc = _CACHE[key]
    in_maps = host_prepare(inputs, cfg)
    res = bass_utils.run_bass_kernel_spmd(
        nc, in_maps, core_ids=list(range(cfg.NC)))
    return np.concatenate([res.results[c]["out"] for c in range(cfg.NC)], axis=0)


if __name__ == "__main__":
    rng = np.random.default_rng(0)
    S2, B = 32768, 256
    demo = {
        "input": rng.standard_normal((B, S2), dtype=np.float32),
        "W_c1": rng.standard_normal((HID, S2), dtype=np.float32) / np.sqrt(S2),
        "b_c1": rng.standard_normal(HID).astype(np.float32) / np.sqrt(S2),
        "W_c2": rng.standard_normal((1, HID), dtype=np.float32) / np.sqrt(HID),
        "W1": rng.standard_normal((N1, S2), dtype=np.float32) / np.sqrt(S2),
        "b1": rng.standard_normal(N1).astype(np.float32) / np.sqrt(S2),
        "W2": rng.standard_normal((HID, N1), dtype=np.float32) / np.sqrt(N1),
        "b2": rng.standard_normal(HID).astype(np.float32) / np.sqrt(N1),
        "W3": rng.standard_normal((N3, HID), dtype=np.float32) / np.sqrt(HID),
        "b3": rng.standard_normal(N3).astype(np.float32) / np.sqrt(HID),
        "W4": rng.standard_normal((N3, N3), dtype=np.float32) / np.sqrt(N3),
    }
    out = kernel(**demo)
    print(out.shape, out.dtype, np.abs(out).max())



# revision 34
# speedup vs baseline: 1.1264x; 1.1264x over previous
"""Trainium2 Bass kernel for nn_NeuralNetwork_86990267613505 (topk_masking).

Network (per reference):
  cx = sigmoid(tanh(input @ W_c1.T + b_c1) @ W_c2.T)          # [B] gate
  x  = kwta(input @ W1.T + b1, k=int(cx*1024))                # [B,1024]
  x  = kwta(x @ W2.T + b2,     k=int(cx*512))                 # [B,512]
  x  = kwta(x @ W3.T + b3,     k=int(cx*1024))                # [B,1024]
  out = x @ W4.T                                              # [B,1024]

Sharding: the two big matmuls (contraction over S2=32768) are column-sharded
over the contraction dim across 8 cores (4096 each); partial sums are combined
with a single fused ReduceScatter of [B, 512+1024] which also distributes the
batch (32 rows per core).  Everything after is data-parallel per core.

kwta: per-row exact k-th-largest via 7-pass radix-5 bisection (probes
replicated 4x across partitions; the per-pass replica-sum of count>=k runs
as a PE matmul against a width-scaled [p==q mod 32] matrix, which sums and
broadcasts in one instruction), then band extraction (bf16 compares, f32
values) + one max8 + indicator-pick, then mask = (x >= thresh) * x.

Precision strategy (the kWTA selection cliff needs ~1e-5 absolute accuracy
on every pre-kwta activation): the streamed matmuls use a 3-pass bf16 hi/lo
decomposition (hi*hi + hi*lo + lo*hi), biases are folded in as a K=2 matmul
of hi/lo bias rows, and the tail matmuls stay fp32 except the final output
matmul which runs float32r (its ~2^-12 noise does not pass any selection).
"""

import numpy as np

import concourse.bacc as bacc
import concourse.mybir as mybir
import concourse.tile as tile
from concourse import bass_utils

F32 = mybir.dt.float32
F32R = mybir.dt.float32r
BF16 = mybir.dt.bfloat16
I32 = mybir.dt.int32
I16 = mybir.dt.int16
ALU = mybir.AluOpType
ACTF = mybir.ActivationFunctionType

HID = 512
N1 = 2 * HID      # 1024
N3 = 1024         # HEADS
R = 32            # rows per core after scatter
C = 4             # partition replication for probing
BIG = 1e30
N_PASS = 7        # radix-5 bisection passes (top-8 band needs >=7)


class Cfg:
    def __init__(self, S2=32768, B=256, NC=8, chunk=4, debug=False):
        assert B // NC == R
        self.S2, self.B, self.NC, self.chunk = S2, B, NC, chunk
        self.debug = debug
        self.no_collective = False
        self.loop_n = 0
        self.phase = None  # None | 'notail'
        self.pe_reduce = True   # PE-matmul replica reduction in kwta
        self.warmup = False     # PE pre-warm (helps one-shot only)
        self.KSH = S2 // NC            # contraction shard per core
        self.KT = self.KSH // 128      # k-tiles
        assert self.KT % chunk == 0
        self.SW = B + 3 * HID          # stream free width per k-tile
        # b-tiles: chunks of <=128 rows of the full batch
        self.b_tiles = [(s, min(128, B - s)) for s in range(0, B, 128)]


def _floorize3(nc, sb, val_ap, rows, name):
    """floor(val) per column for val >= 0 on [rows,3], HW float->int is RNE."""
    ki = sb.tile([rows, 3], I32, name=f"{name}_i")
    kb = sb.tile([rows, 3], F32, name=f"{name}_b")
    cmp = sb.tile([rows, 3], F32, name=f"{name}_c")
    kf = sb.tile([rows, 3], F32, name=f"{name}_f")
    nc.vector.tensor_copy(ki[:], val_ap)
    nc.vector.tensor_copy(kb[:], ki[:])
    nc.vector.tensor_tensor(cmp[:], kb[:], val_ap, ALU.is_gt)
    nc.vector.tensor_sub(kf[:], kb[:], cmp[:])
    return kf


def _pe_keepalive(nc, ps, src_ap, lname, i):
    pdum = ps.tile([1, 1], F32, tag="tp", bufs=2, name=f"{lname}_pd{i}")
    nc.tensor.matmul(pdum[:], src_ap, src_ap, start=True, stop=True)


def _kwta(nc, sb, ps, x_ap, ktile, kcol, n, consts, lname, pe_reduce=True):
    """x_ap: [128, n] fp32 SBUF (rows replicated 4x: partition 32c+r = row r).
    ktile[:, kcol]: [128,1] fp32 float(k).  Returns masked [R, n] f32 tile.

    Radix-5 bisection; the per-pass replica-sum of the count>=k indicator is
    done on the PE (matmul with a [p==q mod 32] 0/1 matrix), which both sums
    over the 4 replicas and broadcasts the result to all 128 partitions in a
    single instruction, replacing a 6-op DVE shuffle chain.
    """
    iota16 = consts["iota16"]
    fpw, rmatw = consts["fpw"], consts["rmatw"]

    # lp[:,0] = lo, lp[:,1] = probe.  Init const: lo=-16, probe = frac*32-16.
    lp = consts["lp0"]

    xb = sb.tile([128, n], BF16, tag="kw_xb", name=f"{lname}_xb0")
    nc.vector.tensor_copy(xb[:], x_ap)
    trash = sb.tile([128, n], BF16, tag="kw_tr", name=f"{lname}_tr0")
    cnt = sb.tile([128, 1], F32, tag="kw_cnt", bufs=2, name=f"{lname}_cnt0")
    bandv = sb.tile([R, n], F32, tag="kw_bv", name=f"{lname}_bv0")
    nc.vector.memset(bandv[:], -BIG)
    for p in range(N_PASS):
        # probes/lo update fused: lp_new = fpw[p] + w5*j + lo, with the w5
        # scaling folded into the rmatw[p] matmul weights
        nc.vector.tensor_scalar(
            trash[:], xb[:], lp[:, 1:2], None, ALU.is_ge, ALU.add,
            accum_out=cnt[:],
        )
        ge = sb.tile([128, 1], F32, tag="kw_ge", bufs=2, name=f"{lname}_ge{p}")
        nc.vector.tensor_scalar(ge[:], cnt[:], ktile[:, kcol:kcol + 1], None, ALU.is_ge)
        if pe_reduce:
            jp = ps.tile([128, 1], F32, tag="tp", bufs=2, name=f"{lname}_jp{p}")
            nc.tensor.matmul(jp[:], rmatw[:, 128 * p:128 * (p + 1)], ge[:],
                             start=True, stop=True)
            j_ap = jp[:, 0:1]
        else:
            sh64 = sb.tile([64, 1], F32, tag="kw_s64", bufs=2, name=f"{lname}_s64_{p}")
            f2 = sb.tile([64, 1], F32, tag="kw_f2", bufs=2, name=f"{lname}_f2_{p}")
            sh32 = sb.tile([32, 1], F32, tag="kw_s32", bufs=2, name=f"{lname}_s32_{p}")
            jall = sb.tile([128, 1], F32, tag="kw_j", bufs=2, name=f"{lname}_j{p}")
            nc.vector.tensor_copy(sh64[:], ge[64:128, :])
            nc.vector.tensor_add(f2[:], ge[0:64, :], sh64[:])
            nc.vector.tensor_copy(sh32[:], f2[32:64, :])
            nc.vector.tensor_add(jall[0:32, :], f2[0:32, :], sh32[:])
            nc.vector.tensor_copy(jall[32:64, :], jall[0:32, :])
            nc.vector.tensor_copy(jall[64:128, :], jall[0:64, :])
            jsc = sb.tile([128, 1], F32, tag="kw_jsc", bufs=2, name=f"{lname}_jsc{p}")
            nc.vector.tensor_scalar(jsc[:], jall[:], 32.0 * 0.2 ** (p + 1),
                                    None, ALU.mult)
            j_ap = jsc[:, 0:1]
            _pe_keepalive(nc, ps, ge[0:1, 0:1], lname, p)
        lp_new = sb.tile([128, 2], F32, tag="kw_lp", bufs=2, name=f"{lname}_lp{p+1}")
        nc.vector.tensor_scalar(lp_new[:], fpw[:, 2 * p:2 * (p + 1)],
                                j_ap, lp[:, 0:1], ALU.add, ALU.add)
        lp = lp_new

    lo = lp[:, 0:1]
    w_fin = 32.0 * 0.2 ** N_PASS
    hi = sb.tile([128, 1], F32, name=f"{lname}_hi")
    nc.vector.tensor_scalar(hi[:], lo, w_fin, None, ALU.add)
    # band membership from bf16 compares (monotone rounding keeps the true
    # k-th inside), band VALUES from exact f32 x.
    x_r = x_ap[0:R, :]
    bhi = sb.tile([R, n], BF16, tag="kw_bhi", name=f"{lname}_bhi0")
    binb = sb.tile([R, n], I16, tag="kw_binb", name=f"{lname}_binb0")
    nc.vector.tensor_scalar(bhi[:], xb[0:R, :], hi[0:R, 0:1], None, ALU.is_lt)
    nc.vector.scalar_tensor_tensor(
        binb[:], xb[0:R, :], lo[0:R, 0:1], bhi[:], ALU.is_ge, ALU.mult)
    # c_hi = count(x >= hi); issued after the band compares so the
    # band->max8 critical path advances first (chi is only needed at pick)
    chi = sb.tile([128, 1], F32, name=f"{lname}_chi")
    nc.vector.tensor_scalar(
        trash[:], xb[:], hi[:, 0:1], None, ALU.is_ge, ALU.add, accum_out=chi[:],
    )
    _pe_keepalive(nc, ps, hi[0:1, 0:1], lname, "ka")
    nc.vector.copy_predicated(bandv[:], binb[:], x_r)
    # top-8 of band (band width 0.002 keeps expected band size ~1.5)
    m8 = sb.tile([R, 8], F32, name=f"{lname}_m8")
    nc.vector.max(m8[:], bandv[:])
    # pick (k - c_hi - 1)-th
    rf = sb.tile([R, 1], F32, name=f"{lname}_rf")
    nc.vector.tensor_sub(rf[:], ktile[0:R, kcol:kcol + 1], chi[0:R, :])
    nc.vector.tensor_scalar(rf[:], rf[:], 1.0, None, ALU.subtract)
    nc.vector.tensor_scalar(rf[:], rf[:], 0.0, 7.0, ALU.max, ALU.min)
    ind = sb.tile([R, 8], F32, name=f"{lname}_ind")
    nc.vector.tensor_scalar(ind[:], iota16[0:R, 0:8], rf[:, 0:1], None, ALU.is_equal)
    iv = sb.tile([R, 8], F32, name=f"{lname}_iv")
    nc.vector.tensor_mul(iv[:], ind[:], m8[:])
    vk = sb.tile([R, 1], F32, name=f"{lname}_vk")
    nc.vector.reduce_sum(vk[:], iv[:], axis=mybir.AxisListType.X)
    # thresh = k>=1 ? vk : +BIG
    g = sb.tile([R, 1], F32, name=f"{lname}_g")
    ga = sb.tile([R, 1], F32, name=f"{lname}_ga")
    gb = sb.tile([R, 1], F32, name=f"{lname}_gb")
    thr = sb.tile([R, 1], F32, name=f"{lname}_thr")
    nc.vector.tensor_scalar(g[:], ktile[0:R, kcol:kcol + 1], 1.0, None, ALU.is_ge)
    nc.vector.tensor_scalar(ga[:], g[:], -BIG, BIG, ALU.mult, ALU.add)
    nc.vector.tensor_scalar(gb[:], g[:], vk[:, 0:1], None, ALU.mult)
    nc.vector.tensor_add(thr[:], ga[:], gb[:])
    # masked = (x >= thresh) * x
    masked = sb.tile([R, n], F32, tag="kw_mask", name=f"{lname}_masked")
    nc.vector.scalar_tensor_tensor(
        masked[:], x_r, thr[:, 0:1], x_r, ALU.is_ge, ALU.mult)
    return masked, thr


def _transpose_chunks(nc, sb, pst, masked, n, ident, rep, lname, dtype=F32):
    """masked [R, n] f32 -> list of xT tiles: [128, C*R] (rep) or [128, R]."""
    tiles = []
    for ch in range(n // 128):
        pt = pst.tile([128, R], F32, tag="tp", name=f"{lname}_pt{ch}")
        nc.tensor.transpose(pt[:], masked[:, 128 * ch:128 * (ch + 1)],
                            ident[0:R, 0:R])
        if rep:
            xt = sb.tile([128, C * R], dtype, tag="kw_xt", bufs=8,
                         name=f"{lname}_xt{ch}")
            nc.vector.tensor_copy(
                xt[:].rearrange("p (c r) -> p c r", c=C),
                pt[:, :].unsqueeze(1).broadcast_to([128, C, R]),
            )
        else:
            xt = sb.tile([128, R], dtype, tag="kw_xt", bufs=8,
                         name=f"{lname}_xt{ch}")
            nc.vector.tensor_copy(xt[:], pt[:])
        tiles.append(xt)
    return tiles


def build_nc(cfg: Cfg):
    nc = bacc.Bacc("TRN2", target_bir_lowering=False, debug=False,
                   num_devices=cfg.NC)
    B, NC, KT, SW, chunk = cfg.B, cfg.NC, cfg.KT, cfg.SW, cfg.chunk

    stream_d = nc.dram_tensor("stream", [KT, 128, 2, SW], BF16, kind="ExternalInput")
    ident_d = nc.dram_tensor("ident", [128, 128], F32, kind="ExternalInput")
    brow_d = nc.dram_tensor("brow", [2, 3 * HID], BF16, kind="ExternalInput")
    b2rep_d = nc.dram_tensor("b2rep", [128, HID], F32, kind="ExternalInput")
    b3rep_d = nc.dram_tensor("b3rep", [128, N3], F32, kind="ExternalInput")
    wc2rep_d = nc.dram_tensor("wc2rep", [128, HID], F32, kind="ExternalInput")
    fracpair_d = nc.dram_tensor("fracpair", [128, 2], F32, kind="ExternalInput")
    lp0_d = nc.dram_tensor("lp0", [128, 2], F32, kind="ExternalInput")
    fpw_d = nc.dram_tensor("fpw", [N_PASS, 128, 2], F32, kind="ExternalInput")
    rmat_d = nc.dram_tensor("rmat", [128, 128], F32, kind="ExternalInput")
    rmatw_d = nc.dram_tensor("rmatw", [N_PASS, 128, 128], F32, kind="ExternalInput")
    kmul_d = nc.dram_tensor("kmul", [R, 3], F32, kind="ExternalInput")
    iota16_d = nc.dram_tensor("iota16", [R, 16], F32, kind="ExternalInput")
    w2t_d = nc.dram_tensor("w2t", [N1, HID], F32, kind="ExternalInput")
    w3t_d = nc.dram_tensor("w3t", [HID, N3], F32R, kind="ExternalInput")
    w4t_d = nc.dram_tensor("w4t", [N3, N3], F32R, kind="ExternalInput")
    out_d = nc.dram_tensor("out", [R, N3], F32, kind="ExternalOutput")
    if cfg.debug:
        dbg_rs_d = nc.dram_tensor("dbg_rs", [R, 3 * HID], F32, kind="ExternalOutput")
        dbg_gate_d = nc.dram_tensor("dbg_gate", [R, 8], F32, kind="ExternalOutput")
        dbg_x2_d = nc.dram_tensor("dbg_x2", [R, HID], F32, kind="ExternalOutput")
        dbg_x3_d = nc.dram_tensor("dbg_x3", [R, N3], F32, kind="ExternalOutput")

    import contextlib
    with tile.TileContext(nc) as tc:
        loop_ctx = tc.For_i(0, cfg.loop_n, 1) if cfg.loop_n else contextlib.nullcontext()
        with (
            loop_ctx,
            tc.tile_pool(name="consts", bufs=1) as cp,
            tc.tile_pool(name="stream", bufs=2) as sp,
            tc.tile_pool(name="acc", bufs=1, space="PSUM") as ap,
            tc.tile_pool(name="sb", bufs=1) as sb,
            tc.tile_pool(name="pst", bufs=2, space="PSUM") as pst,
            tc.tile_pool(name="dram", bufs=1, space="DRAM") as dram,
        ):
            # ---- constants ----
            ident = cp.tile([128, 128], F32, name="ident")
            brow = cp.tile([2, 3 * HID], BF16, name="brow")
            ones2 = cp.tile([2, 128], BF16, name="ones2")
            b2rep = cp.tile([128, HID], F32, name="b2rep")
            b3rep = cp.tile([128, N3], F32, name="b3rep")
            wc2rep = cp.tile([128, HID], F32, name="wc2rep")
            fracpair = cp.tile([128, 2], F32, name="fracpair")
            lp0 = cp.tile([128, 2], F32, name="lp0")
            fpw = cp.tile([128, N_PASS * 2], F32, name="fpw")
            rmat = cp.tile([128, 128], F32, name="rmat")
            rmatw = cp.tile([128, N_PASS * 128], F32, name="rmatw")
            kmul = cp.tile([R, 3], F32, name="kmul")
            iota16 = cp.tile([R, 16], F32, name="iota16")
            nc.sync.dma_start(ident[:], ident_d.ap())
            nc.sync.dma_start(brow[:], brow_d.ap())
            nc.vector.memset(ones2[:], 1.0)
            nc.sync.dma_start(b2rep[:], b2rep_d.ap())
            nc.sync.dma_start(b3rep[:], b3rep_d.ap())
            nc.sync.dma_start(wc2rep[:], wc2rep_d.ap())
            nc.sync.dma_start(fracpair[:], fracpair_d.ap())
            nc.sync.dma_start(lp0[:], lp0_d.ap())
            nc.sync.dma_start(fpw[:].rearrange("p (n t) -> p n t", n=N_PASS),
                              fpw_d.ap().transpose([1, 0, 2]))
            nc.sync.dma_start(rmatw[:].rearrange("p (n q) -> p n q", n=N_PASS),
                              rmatw_d.ap().transpose([1, 0, 2]))
            nc.sync.dma_start(rmat[:], rmat_d.ap())
            nc.sync.dma_start(kmul[:], kmul_d.ap())
            nc.sync.dma_start(iota16[:], iota16_d.ap())
            consts = {"ident": ident, "fracpair": fracpair, "rmat": rmat,
                      "lp0": lp0, "fpw": fpw, "rmatw": rmatw,
                      "iota16": iota16}

            # ---- phase A: streamed big matmuls ----
            # one wide accumulator [bsz, 3*HID] per b_tile: each (pass, btile)
            # is a single matmul, sharing one stationary-weight load
            pacc = {}
            for bi, (bs, bsz) in enumerate(cfg.b_tiles):
                pacc[bi] = ap.tile([bsz, 3 * HID], F32, tag="acc", bufs=2,
                                   name=f"pacc{bi}")

            # PE p-state pre-warm: dummy matmuls on ident while the first
            # stream chunk is still in flight, so real matmuls start at the
            # max clock instead of spending their first ~3us ramping.
            if cfg.warmup:
                for wi in range(16):
                    warm = pst.tile([128, 128], F32, tag="tp", bufs=2,
                                    name=f"warm{wi}")
                    nc.tensor.matmul(warm[:], ident[:], ident[:, 0:128],
                                     start=True, stop=True)

            n_chunks = KT // chunk
            for cki in range(n_chunks):
                st = sp.tile([128, chunk * 2 * SW], BF16, tag="st", name=f"st{cki}")
                src = stream_d.ap()[chunk * cki: chunk * (cki + 1)]
                nc.sync.dma_start(
                    st[:].rearrange("p (c t w) -> p c t w", c=chunk, t=2),
                    src.transpose([1, 0, 2, 3]),
                )
                for ki in range(chunk):
                    kt = chunk * cki + ki
                    hi = st[:, (2 * ki) * SW:(2 * ki + 1) * SW]
                    lo = st[:, (2 * ki + 1) * SW:(2 * ki + 2) * SW]
                    first = kt == 0
                    # group the two hi-lhsT passes (hi*hi, hi*lo) under one
                    # stationary-weight load per b_tile; lo-lhsT pass last
                    for bi, (bs, bsz) in enumerate(cfg.b_tiles):
                        lhsT = hi[:, bs:bs + bsz]
                        for pi, wb in enumerate((hi, lo)):
                            f = first and pi == 0
                            for o in range(3):
                                nc.tensor.matmul(
                                    pacc[bi][:, HID * o:HID * (o + 1)], lhsT,
                                    wb[:, B + HID * o:B + HID * (o + 1)],
                                    start=f, stop=False)
                    for bi, (bs, bsz) in enumerate(cfg.b_tiles):
                        lhsT = lo[:, bs:bs + bsz]
                        for o in range(3):
                            nc.tensor.matmul(
                                pacc[bi][:, HID * o:HID * (o + 1)], lhsT,
                                hi[:, B + HID * o:B + HID * (o + 1)],
                                start=False, stop=False)
            # bias add via a K=2 matmul of ones against the hi/lo bias rows
            # (closes each accumulation group)
            for bi, (bs, bsz) in enumerate(cfg.b_tiles):
                for o in range(3):
                    nc.tensor.matmul(pacc[bi][:, HID * o:HID * (o + 1)],
                                     ones2[:, 0:bsz],
                                     brow[:, HID * o:HID * (o + 1)],
                                     start=False, stop=True)

            # ---- tail weights (after the stream so they don't delay it) ----
            w2sb = cp.tile([128, 8 * HID], F32, name="w2sb")
            w3sb = cp.tile([128, 4 * N3], F32R, name="w3sb")
            w4sb = cp.tile([128, 8 * N3], F32R, name="w4sb")
            nc.sync.dma_start(
                w2sb[:].rearrange("p (c w) -> p c w", c=8),
                w2t_d.ap().rearrange("(c p) w -> p c w", p=128))
            nc.sync.dma_start(
                w3sb[:].rearrange("p (c w) -> p c w", c=4),
                w3t_d.ap().rearrange("(c p) w -> p c w", p=128))
            nc.sync.dma_start(
                w4sb[:].rearrange("p (c w) -> p c w", c=8),
                w4t_d.ap().rearrange("(c p) w -> p c w", p=128))

            # ---- phase B: stage PSUM->SBUF (bias already added in-PE), RS ----
            rs_in = dram.tile([B, 3 * HID], F32, name="rs_in")
            rs_out = dram.tile([R, 3 * HID], F32, name="rs_out")
            for bi, (bs, bsz) in enumerate(cfg.b_tiles):
                so = sb.tile([bsz, 3 * HID], F32, tag="rsin_sb", bufs=2,
                             name=f"so{bi}")
                if bi == 0:
                    nc.vector.tensor_copy(so[:], pacc[bi][:])
                else:
                    nc.scalar.activation(so[:], pacc[bi][:], ACTF.Identity)
                nc.sync.dma_start(rs_in[bs:bs + bsz, :], so[:])
            if cfg.no_collective:
                nc.sync.dma_start(rs_out[:], rs_in[0:R, :])
            else:
                nc.gpsimd.collective_compute(
                    "ReduceScatter", ALU.add,
                    replica_groups=[list(range(NC))],
                    ins=[rs_in.opt()], outs=[rs_out.opt()],
                )

            # ---- phase C: replicated load + gate ----
            # replica 0 carries the full row (gate cols + x1); replicas 1-3
            # only need the x1 columns for the kwta probe replication.
            xall = sb.tile([128, 3 * HID], F32, name="xall")
            nc.sync.dma_start(xall[0:R, :], rs_out[:])
            for c in range(1, C):
                nc.sync.dma_start(xall[c * R:(c + 1) * R, HID:3 * HID],
                                  rs_out[:, HID:3 * HID])
            th = sb.tile([R, HID], F32, name="tanh")
            nc.scalar.activation(th[:], xall[0:R, 0:HID], ACTF.Tanh)
            ztr = sb.tile([R, HID], F32, name="ztr")
            z = sb.tile([R, 1], F32, name="z")
            nc.vector.scalar_tensor_tensor(
                ztr[:], th[:], 1.0, wc2rep[0:R, :], ALU.mult, ALU.mult,
                accum_out=z[:])
            ez = sb.tile([R, 1], F32, name="ez")
            nc.scalar.activation(ez[:], z[:], ACTF.Exp, scale=-1.0)
            ez1 = sb.tile([R, 1], F32, name="ez1")
            nc.vector.tensor_scalar(ez1[:], ez[:], 1.0, None, ALU.add)
            cx = sb.tile([R, 1], F32, name="cx")
            nc.vector.reciprocal(cx[:], ez1[:])
            kk = sb.tile([R, 3], F32, name="kk")
            nc.vector.tensor_scalar(kk[:], kmul[:], cx[:, 0:1], None, ALU.mult)
            kf3 = _floorize3(nc, sb, kk[:], R, "kf")
            # replicate the three k's to all 128 partitions via the PE
            krep_ps = pst.tile([128, 3], F32, tag="tp", bufs=2, name="krep_ps")
            nc.tensor.matmul(krep_ps[:], rmat[0:R, :], kf3[:], start=True, stop=True)
            krep = sb.tile([128, 3], F32, name="krep")
            nc.vector.tensor_copy(krep[:], krep_ps[:])
            
            do_tail = cfg.phase != "notail"
            if not do_tail:
                nt = sb.tile([R, N3], F32, name="nt")
                nc.vector.tensor_copy(nt[:], xall[0:R, HID:3 * HID])
                nc.vector.tensor_add(nt[:, 0:1], w2sb[0:R, 0:1], w2sb[0:R, 2:3])
                nc.vector.tensor_add(nt[:, 1:2], w2sb[0:R, 1:2], cx[0:R, :])
                nc.vector.tensor_add(nt[:, 2:3], krep[0:R, 0:1], krep[0:R, 1:2])
                nc.vector.tensor_add(nt[:, 3:4], krep[0:R, 2:3], fracpair[0:R, 1:2])
                nc.vector.tensor_add(nt[:, 4:5], ident[0:R, 0:1], iota16[0:R, 0:1])
                nc.vector.tensor_add(nt[:, 5:6], b2rep[0:R, 0:1], b3rep[0:R, 0:1])
                nc.sync.dma_start(out_d.ap(), nt[:])

            if do_tail:
                # ---- layer 1 kwta + mm2 ----
                x1 = xall[:, HID:3 * HID]
                masked1, thr1 = _kwta(nc, sb, pst, x1, krep, 0, N1, consts, "L1", cfg.pe_reduce)
                xt1 = _transpose_chunks(nc, sb, pst, masked1, N1, ident, True, "L1")
                px2 = pacc[0][:, 0:HID]
                w2v = w2sb[:].rearrange("p (c w) -> p c w", c=8)
                for ch in range(8):
                    nc.tensor.matmul(px2, xt1[ch][:], w2v[:, ch, :],
                                     start=(ch == 0), stop=(ch == 7))
                x2 = sb.tile([128, HID], F32, name="x2")
                nc.vector.tensor_add(x2[:], px2, b2rep[:])

                # ---- layer 2 kwta + mm3 ----
                masked2, thr2 = _kwta(nc, sb, pst, x2[:], krep, 1, HID, consts, "L2", cfg.pe_reduce)
                xt2 = _transpose_chunks(nc, sb, pst, masked2, HID, ident, True, "L2",
                                        dtype=F32R)
                px3 = pacc[0][:, HID:HID + N3]
                w3v = w3sb[:].rearrange("p (c w) -> p c w", c=4)
                for ch in range(4):
                    for o in range(2):
                        nc.tensor.matmul(
                            pacc[0][:, HID + 512 * o:HID + 512 * (o + 1)],
                            xt2[ch][:], w3v[:, ch, 512 * o:512 * (o + 1)],
                            start=(ch == 0), stop=(ch == 3))
                x3 = sb.tile([128, N3], F32, name="x3")
                nc.vector.tensor_add(x3[:], px3, b3rep[:])

                # ---- layer 3 kwta + mm4 ----
                masked3, thr3 = _kwta(nc, sb, pst, x3[:], krep, 2, N3, consts, "L3", cfg.pe_reduce)
                xt3 = _transpose_chunks(nc, sb, pst, masked3, N3, ident, False, "L3", dtype=F32R)
                px4 = pacc[1][0:R, 0:N3]
                w4v = w4sb[:].rearrange("p (c w) -> p c w", c=8)
                for ch in range(8):
                    for o in range(2):
                        nc.tensor.matmul(
                            pacc[1][0:R, 512 * o:512 * (o + 1)], xt3[ch][:],
                            w4v[:, ch, 512 * o:512 * (o + 1)],
                            start=(ch == 0), stop=(ch == 7))
                outsb = sb.tile([R, N3], F32, name="outsb")
                nc.vector.tensor_copy(outsb[:, 0:512], pacc[1][0:R, 0:512])
                nc.scalar.activation(outsb[:, 512:1024], pacc[1][0:R, 512:1024],
                                     ACTF.Identity)
                nc.sync.dma_start(out_d.ap(), outsb[:])
                if cfg.debug:
                    nc.sync.dma_start(dbg_rs_d.ap(), xall[0:R, :])
                    gsb = sb.tile([R, 8], F32, name="gsb")
                    nc.vector.tensor_copy(gsb[:, 0:1], cx[:])
                    nc.vector.tensor_copy(gsb[:, 1:2], krep[0:R, 0:1])
                    nc.vector.tensor_copy(gsb[:, 2:3], krep[0:R, 1:2])
                    nc.vector.tensor_copy(gsb[:, 3:4], krep[0:R, 2:3])
                    nc.vector.tensor_copy(gsb[:, 4:5], thr1[:])
                    nc.vector.tensor_copy(gsb[:, 5:6], thr2[:])
                    nc.vector.tensor_copy(gsb[:, 6:7], thr3[:])
                    nc.vector.tensor_copy(gsb[:, 7:8], z[:])
                    nc.sync.dma_start(dbg_gate_d.ap(), gsb[:])
                    nc.sync.dma_start(dbg_x2_d.ap(), x2[0:R, :])
                    nc.sync.dma_start(dbg_x3_d.ap(), x3[0:R, :])

    nc.compile()
    return nc


def host_prepare(inputs, cfg: Cfg):
    """Build per-core in_maps from the full inputs."""
    B, NC, KT, SW, KSH = cfg.B, cfg.NC, cfg.KT, cfg.SW, cfg.KSH
    f32 = np.float32
    inp = np.asarray(inputs["input"], f32)
    W_c1 = np.asarray(inputs["W_c1"], f32)
    b_c1 = np.asarray(inputs["b_c1"], f32)
    W_c2 = np.asarray(inputs["W_c2"], f32)
    W1 = np.asarray(inputs["W1"], f32)
    b1 = np.asarray(inputs["b1"], f32)
    W2 = np.asarray(inputs["W2"], f32)
    b2 = np.asarray(inputs["b2"], f32)
    W3 = np.asarray(inputs["W3"], f32)
    b3 = np.asarray(inputs["b3"], f32)
    W4 = np.asarray(inputs["W4"], f32)

    xT = np.ascontiguousarray(inp.T)          # [S2, B]
    wc1T = np.ascontiguousarray(W_c1.T)       # [S2, HID]
    w1T = np.ascontiguousarray(W1.T)          # [S2, N1]

    import ml_dtypes as _mld
    bcat = np.concatenate([b_c1, b1]).astype(f32) / NC
    bhi = bcat.astype(_mld.bfloat16)
    blo = (bcat - bhi.astype(f32)).astype(_mld.bfloat16)
    frac_col = ((np.arange(128, dtype=f32) // R + 1.0) / 5.0).astype(f32)
    pp = np.arange(128)
    consts = {
        "ident": np.eye(128, dtype=f32),
        "brow": np.stack([bhi, blo], axis=0),
        "b2rep": np.broadcast_to(b2, (128, HID)).copy(),
        "b3rep": np.broadcast_to(b3, (128, N3)).copy(),
        "wc2rep": np.broadcast_to(W_c2[0], (128, HID)).copy(),
        "fracpair": np.stack([np.zeros(128, f32), frac_col], axis=1).copy(),
        "lp0": np.stack([np.full(128, -16.0, f32),
                         (frac_col * 32.0 - 16.0).astype(f32)], axis=1).copy(),
        "fpw": np.stack([np.stack([np.zeros(128, f32),
                                   (frac_col * (32.0 * 0.2 ** (p + 1))).astype(f32)],
                                  axis=1)
                         for p in range(7)], axis=0).copy(),
        "rmat": (pp[:, None] % R == pp[None, :] % R).astype(f32),
        "rmatw": np.stack([(pp[:, None] % R == pp[None, :] % R).astype(f32)
                           * np.float32(32.0 * 0.2 ** (p + 1))
                           for p in range(7)], axis=0).copy(),
        "kmul": np.broadcast_to(
            np.array([2 * HID, HID, N3], f32), (R, 3)).copy(),
        "iota16": np.broadcast_to(np.arange(16, dtype=f32), (R, 16)).copy(),
        "w2t": np.ascontiguousarray(W2.T),
        "w3t": np.ascontiguousarray(W3.T),
        "w4t": np.ascontiguousarray(W4.T),
    }

    import ml_dtypes
    bf16 = ml_dtypes.bfloat16
    in_maps = []
    for c in range(NC):
        sl = slice(c * KSH, (c + 1) * KSH)
        stream = np.concatenate([xT[sl], wc1T[sl], w1T[sl]], axis=1)  # [KSH, SW]
        hi = stream.astype(bf16)
        lo = (stream - hi.astype(f32)).astype(bf16)
        shl = np.stack([hi, lo], axis=1).reshape(KSH, 2, cfg.SW)  # [KSH,2,SW]
        shl = np.ascontiguousarray(shl.reshape(KT, 128, 2, cfg.SW))
        in_maps.append({"stream": shl, **consts})
    return in_maps


_CACHE = {}


def kernel(**inputs) -> np.ndarray:
    cfg = Cfg(S2=inputs["input"].shape[1], B=inputs["input"].shape[0])
    key = (cfg.S2, cfg.B, cfg.NC)
    if key not in _CACHE:
        _CACHE[key] = build_nc(cfg)
    nc = _CACHE[key]
    in_maps = host_prepare(inputs, cfg)
    res = bass_utils.run_bass_kernel_spmd(
        nc, in_maps, core_ids=list(range(cfg.NC)))
    return np.concatenate([res.results[c]["out"] for c in range(cfg.NC)], axis=0)


if __name__ == "__main__":
    rng = np.random.default_rng(0)
    S2, B = 32768, 256
    demo = {
        "input": rng.standard_normal((B, S2), dtype=np.float32),
        "W_c1": rng.standard_normal((HID, S2), dtype=np.float32) / np.sqrt(S2),
        "b_c1": rng.standard_normal(HID).astype(np.float32) / np.sqrt(S2),
        "W_c2": rng.standard_normal((1, HID), dtype=np.float32) / np.sqrt(HID),
        "W1": rng.standard_normal((N1, S2), dtype=np.float32) / np.sqrt(S2),
        "b1": rng.standard_normal(N1).astype(np.float32) / np.sqrt(S2),
        "W2": rng.standard_normal((HID, N1), dtype=np.float32) / np.sqrt(N1),
        "b2": rng.standard_normal(HID).astype(np.float32) / np.sqrt(N1),
        "W3": rng.standard_normal((N3, HID), dtype=np.float32) / np.sqrt(HID),
        "b3": rng.standard_normal(N3).astype(np.float32) / np.sqrt(HID),
        "W4": rng.standard_normal((N3, N3), dtype=np.float32) / np.sqrt(N3),
    }
    out = kernel(**demo)
    print(out.shape, out.dtype, np.abs(out).max())



# revision 35
# speedup vs baseline: 1.1706x; 1.0392x over previous
"""Trainium2 Bass kernel for nn_NeuralNetwork_86990267613505 (topk_masking).

Network (per reference):
  cx = sigmoid(tanh(input @ W_c1.T + b_c1) @ W_c2.T)          # [B] gate
  x  = kwta(input @ W1.T + b1, k=int(cx*1024))                # [B,1024]
  x  = kwta(x @ W2.T + b2,     k=int(cx*512))                 # [B,512]
  x  = kwta(x @ W3.T + b3,     k=int(cx*1024))                # [B,1024]
  out = x @ W4.T                                              # [B,1024]

Sharding: the two big matmuls (contraction over S2=32768) are column-sharded
over the contraction dim across 8 cores (4096 each); partial sums are combined
with a single fused ReduceScatter of [B, 512+1024] which also distributes the
batch (32 rows per core).  Everything after is data-parallel per core.

kwta: per-row exact k-th-largest via 7-pass radix-5 bisection (probes
replicated 4x across partitions; the per-pass replica-sum of count>=k runs
as a PE matmul against a width-scaled [p==q mod 32] matrix, which sums and
broadcasts in one instruction), then band extraction (bf16 compares, f32
values) + one max8 + indicator-pick, then mask = (x >= thresh) * x.

Precision strategy (the kWTA selection cliff needs ~1e-5 absolute accuracy
on every pre-kwta activation): the streamed matmuls use a 3-pass bf16 hi/lo
decomposition (hi*hi + hi*lo + lo*hi), biases are folded in as a K=2 matmul
of hi/lo bias rows, and the tail matmuls stay fp32 except the final output
matmul which runs float32r (its ~2^-12 noise does not pass any selection).
"""

import numpy as np

import concourse.bacc as bacc
import concourse.mybir as mybir
import concourse.tile as tile
from concourse import bass_utils

F32 = mybir.dt.float32
F32R = mybir.dt.float32r
BF16 = mybir.dt.bfloat16
I32 = mybir.dt.int32
I16 = mybir.dt.int16
ALU = mybir.AluOpType
ACTF = mybir.ActivationFunctionType

HID = 512
N1 = 2 * HID      # 1024
N3 = 1024         # HEADS
R = 32            # rows per core after scatter
C = 4             # partition replication for probing
BIG = 1e30
N_PASS = 7        # radix-5 bisection passes (top-8 band needs >=7)


class Cfg:
    def __init__(self, S2=32768, B=256, NC=8, chunk=4, debug=False):
        assert B // NC == R
        self.S2, self.B, self.NC, self.chunk = S2, B, NC, chunk
        self.debug = debug
        self.no_collective = False
        self.loop_n = 0
        self.phase = None  # None | 'notail'
        self.pe_reduce = True   # PE-matmul replica reduction in kwta
        self.warmup = False     # PE pre-warm (helps one-shot only)
        self.KSH = S2 // NC            # contraction shard per core
        self.KT = self.KSH // 128      # k-tiles
        assert self.KT % chunk == 0
        self.SW = B + 3 * HID          # stream free width per k-tile
        # b-tiles: chunks of <=128 rows of the full batch
        self.b_tiles = [(s, min(128, B - s)) for s in range(0, B, 128)]


def _floorize3(nc, sb, val_ap, rows, name):
    """floor(val) per column for val >= 0 on [rows,3], HW float->int is RNE."""
    ki = sb.tile([rows, 3], I32, name=f"{name}_i")
    kb = sb.tile([rows, 3], F32, name=f"{name}_b")
    cmp = sb.tile([rows, 3], F32, name=f"{name}_c")
    kf = sb.tile([rows, 3], F32, name=f"{name}_f")
    nc.vector.tensor_copy(ki[:], val_ap)
    nc.vector.tensor_copy(kb[:], ki[:])
    nc.vector.tensor_tensor(cmp[:], kb[:], val_ap, ALU.is_gt)
    nc.vector.tensor_sub(kf[:], kb[:], cmp[:])
    return kf


def _pe_keepalive(nc, ps, src_ap, lname, i):
    pdum = ps.tile([1, 1], F32, tag="tp", bufs=2, name=f"{lname}_pd{i}")
    nc.tensor.matmul(pdum[:], src_ap, src_ap, start=True, stop=True)


def _kwta(nc, sb, ps, x_ap, ktile, kcol, n, consts, lname, pe_reduce=True):
    """x_ap: [128, n] fp32 SBUF (rows replicated 4x: partition 32c+r = row r).
    ktile[:, kcol]: [128,1] fp32 float(k).  Returns masked [R, n] f32 tile.

    Radix-5 bisection; the per-pass replica-sum of the count>=k indicator is
    done on the PE (matmul with a [p==q mod 32] 0/1 matrix), which both sums
    over the 4 replicas and broadcasts the result to all 128 partitions in a
    single instruction, replacing a 6-op DVE shuffle chain.
    """
    iota16 = consts["iota16"]
    fpw, rmatw = consts["fpw"], consts["rmatw"]

    # lp[:,0] = lo, lp[:,1] = probe.  Init const: lo=-16, probe = frac*32-16.
    lp = consts["lp0"]

    xb = sb.tile([128, n], BF16, tag="kw_xb", name=f"{lname}_xb0")
    nc.vector.tensor_copy(xb[:], x_ap)
    trash = sb.tile([128, n], BF16, tag="kw_tr", name=f"{lname}_tr0")
    cnt = sb.tile([128, 1], F32, tag="kw_cnt", bufs=2, name=f"{lname}_cnt0")
    bandv = sb.tile([R, n], F32, tag="kw_bv", name=f"{lname}_bv0")
    nc.vector.memset(bandv[:], -BIG)
    for p in range(N_PASS):
        # probes/lo update fused: lp_new = fpw[p] + w5*j + lo, with the w5
        # scaling folded into the rmatw[p] matmul weights
        nc.vector.tensor_scalar(
            trash[:], xb[:], lp[:, 1:2], None, ALU.is_ge, ALU.add,
            accum_out=cnt[:],
        )
        ge = sb.tile([128, 1], F32, tag="kw_ge", bufs=2, name=f"{lname}_ge{p}")
        nc.vector.tensor_scalar(ge[:], cnt[:], ktile[:, kcol:kcol + 1], None, ALU.is_ge)
        if pe_reduce:
            jp = ps.tile([128, 1], F32, tag="tp", bufs=2, name=f"{lname}_jp{p}")
            nc.tensor.matmul(jp[:], rmatw[:, 128 * p:128 * (p + 1)], ge[:],
                             start=True, stop=True)
            j_ap = jp[:, 0:1]
        else:
            sh64 = sb.tile([64, 1], F32, tag="kw_s64", bufs=2, name=f"{lname}_s64_{p}")
            f2 = sb.tile([64, 1], F32, tag="kw_f2", bufs=2, name=f"{lname}_f2_{p}")
            sh32 = sb.tile([32, 1], F32, tag="kw_s32", bufs=2, name=f"{lname}_s32_{p}")
            jall = sb.tile([128, 1], F32, tag="kw_j", bufs=2, name=f"{lname}_j{p}")
            nc.vector.tensor_copy(sh64[:], ge[64:128, :])
            nc.vector.tensor_add(f2[:], ge[0:64, :], sh64[:])
            nc.vector.tensor_copy(sh32[:], f2[32:64, :])
            nc.vector.tensor_add(jall[0:32, :], f2[0:32, :], sh32[:])
            nc.vector.tensor_copy(jall[32:64, :], jall[0:32, :])
            nc.vector.tensor_copy(jall[64:128, :], jall[0:64, :])
            jsc = sb.tile([128, 1], F32, tag="kw_jsc", bufs=2, name=f"{lname}_jsc{p}")
            nc.vector.tensor_scalar(jsc[:], jall[:], 32.0 * 0.2 ** (p + 1),
                                    None, ALU.mult)
            j_ap = jsc[:, 0:1]
            _pe_keepalive(nc, ps, ge[0:1, 0:1], lname, p)
        lp_new = sb.tile([128, 2], F32, tag="kw_lp", bufs=2, name=f"{lname}_lp{p+1}")
        nc.vector.tensor_scalar(lp_new[:], fpw[:, 2 * p:2 * (p + 1)],
                                j_ap, lp[:, 0:1], ALU.add, ALU.add)
        lp = lp_new

    lo = lp[:, 0:1]
    w_fin = 32.0 * 0.2 ** N_PASS
    hi = sb.tile([128, 1], F32, name=f"{lname}_hi")
    nc.vector.tensor_scalar(hi[:], lo, w_fin, None, ALU.add)
    # band membership from bf16 compares (monotone rounding keeps the true
    # k-th inside), band VALUES from exact f32 x.
    x_r = x_ap[0:R, :]
    bhi = sb.tile([R, n], BF16, tag="kw_bhi", name=f"{lname}_bhi0")
    binb = sb.tile([R, n], I16, tag="kw_binb", name=f"{lname}_binb0")
    nc.vector.tensor_scalar(bhi[:], xb[0:R, :], hi[0:R, 0:1], None, ALU.is_lt)
    nc.vector.scalar_tensor_tensor(
        binb[:], xb[0:R, :], lo[0:R, 0:1], bhi[:], ALU.is_ge, ALU.mult)
    # c_hi = count(x >= hi); issued after the band compares so the
    # band->max8 critical path advances first (chi is only needed at pick)
    chi = sb.tile([128, 1], F32, name=f"{lname}_chi")
    nc.vector.tensor_scalar(
        trash[:], xb[:], hi[:, 0:1], None, ALU.is_ge, ALU.add, accum_out=chi[:],
    )
    _pe_keepalive(nc, ps, hi[0:1, 0:1], lname, "ka")
    nc.vector.copy_predicated(bandv[:], binb[:], x_r)
    # top-8 of band (band width 0.002 keeps expected band size ~1.5)
    m8 = sb.tile([R, 8], F32, name=f"{lname}_m8")
    nc.vector.max(m8[:], bandv[:])
    # pick (k - c_hi - 1)-th
    rf = sb.tile([R, 1], F32, name=f"{lname}_rf")
    nc.vector.tensor_sub(rf[:], ktile[0:R, kcol:kcol + 1], chi[0:R, :])
    nc.vector.tensor_scalar(rf[:], rf[:], 1.0, None, ALU.subtract)
    nc.vector.tensor_scalar(rf[:], rf[:], 0.0, 7.0, ALU.max, ALU.min)
    ind = sb.tile([R, 8], F32, name=f"{lname}_ind")
    nc.vector.tensor_scalar(ind[:], iota16[0:R, 0:8], rf[:, 0:1], None, ALU.is_equal)
    iv = sb.tile([R, 8], F32, name=f"{lname}_iv")
    nc.vector.tensor_mul(iv[:], ind[:], m8[:])
    vk = sb.tile([R, 1], F32, name=f"{lname}_vk")
    nc.vector.reduce_sum(vk[:], iv[:], axis=mybir.AxisListType.X)
    # thresh = k>=1 ? vk : +BIG
    g = sb.tile([R, 1], F32, name=f"{lname}_g")
    ga = sb.tile([R, 1], F32, name=f"{lname}_ga")
    gb = sb.tile([R, 1], F32, name=f"{lname}_gb")
    thr = sb.tile([R, 1], F32, name=f"{lname}_thr")
    nc.vector.tensor_scalar(g[:], ktile[0:R, kcol:kcol + 1], 1.0, None, ALU.is_ge)
    nc.vector.tensor_scalar(ga[:], g[:], -BIG, BIG, ALU.mult, ALU.add)
    nc.vector.tensor_scalar(gb[:], g[:], vk[:, 0:1], None, ALU.mult)
    nc.vector.tensor_add(thr[:], ga[:], gb[:])
    # masked = (x >= thresh) * x
    masked = sb.tile([R, n], F32, tag="kw_mask", name=f"{lname}_masked")
    nc.vector.scalar_tensor_tensor(
        masked[:], x_r, thr[:, 0:1], x_r, ALU.is_ge, ALU.mult)
    return masked, thr


def _transpose_chunks(nc, sb, pst, masked, n, ident, rep, lname, dtype=F32):
    """masked [R, n] f32 -> list of xT tiles: [128, C*R] (rep) or [128, R]."""
    tiles = []
    for ch in range(n // 128):
        pt = pst.tile([128, R], F32, tag="tp", name=f"{lname}_pt{ch}")
        nc.tensor.transpose(pt[:], masked[:, 128 * ch:128 * (ch + 1)],
                            ident[0:R, 0:R])
        if rep:
            xt = sb.tile([128, C * R], dtype, tag="kw_xt", bufs=8,
                         name=f"{lname}_xt{ch}")
            nc.vector.tensor_copy(
                xt[:].rearrange("p (c r) -> p c r", c=C),
                pt[:, :].unsqueeze(1).broadcast_to([128, C, R]),
            )
        else:
            xt = sb.tile([128, R], dtype, tag="kw_xt", bufs=8,
                         name=f"{lname}_xt{ch}")
            nc.vector.tensor_copy(xt[:], pt[:])
        tiles.append(xt)
    return tiles


def build_nc(cfg: Cfg):
    nc = bacc.Bacc("TRN2", target_bir_lowering=False, debug=False,
                   num_devices=cfg.NC)
    B, NC, KT, SW, chunk = cfg.B, cfg.NC, cfg.KT, cfg.SW, cfg.chunk

    stream_d = nc.dram_tensor("stream", [KT, 128, 2, SW], BF16, kind="ExternalInput")
    ident_d = nc.dram_tensor("ident", [128, 128], F32, kind="ExternalInput")
    brow_d = nc.dram_tensor("brow", [2, 3 * HID], BF16, kind="ExternalInput")
    b2rep_d = nc.dram_tensor("b2rep", [128, HID], F32, kind="ExternalInput")
    b3rep_d = nc.dram_tensor("b3rep", [128, N3], F32, kind="ExternalInput")
    wc2rep_d = nc.dram_tensor("wc2rep", [128, HID], F32, kind="ExternalInput")
    fracpair_d = nc.dram_tensor("fracpair", [128, 2], F32, kind="ExternalInput")
    lp0_d = nc.dram_tensor("lp0", [128, 2], F32, kind="ExternalInput")
    fpw_d = nc.dram_tensor("fpw", [N_PASS, 128, 2], F32, kind="ExternalInput")
    rmat_d = nc.dram_tensor("rmat", [128, 128], F32, kind="ExternalInput")
    rmatw_d = nc.dram_tensor("rmatw", [N_PASS, 128, 128], F32, kind="ExternalInput")
    kmul_d = nc.dram_tensor("kmul", [R, 3], F32, kind="ExternalInput")
    iota16_d = nc.dram_tensor("iota16", [R, 16], F32, kind="ExternalInput")
    w2t_d = nc.dram_tensor("w2t", [N1, HID], F32, kind="ExternalInput")
    w3t_d = nc.dram_tensor("w3t", [HID, N3], F32R, kind="ExternalInput")
    w4t_d = nc.dram_tensor("w4t", [N3, N3], F32R, kind="ExternalInput")
    out_d = nc.dram_tensor("out", [R, N3], F32, kind="ExternalOutput")
    if cfg.debug:
        dbg_rs_d = nc.dram_tensor("dbg_rs", [R, 3 * HID], F32, kind="ExternalOutput")
        dbg_gate_d = nc.dram_tensor("dbg_gate", [R, 8], F32, kind="ExternalOutput")
        dbg_x2_d = nc.dram_tensor("dbg_x2", [R, HID], F32, kind="ExternalOutput")
        dbg_x3_d = nc.dram_tensor("dbg_x3", [R, N3], F32, kind="ExternalOutput")

    import contextlib
    with tile.TileContext(nc) as tc:
        loop_ctx = tc.For_i(0, cfg.loop_n, 1) if cfg.loop_n else contextlib.nullcontext()
        with (
            tc.tile_pool(name="consts", bufs=1) as cp,
            tc.tile_pool(name="stream", bufs=2) as sp,
            tc.tile_pool(name="acc", bufs=1, space="PSUM") as ap,
            tc.tile_pool(name="sb", bufs=1) as sb,
            tc.tile_pool(name="pst", bufs=2, space="PSUM") as pst,
            tc.tile_pool(name="dram", bufs=1, space="DRAM") as dram,
        ):
            # ---- constants ----
            ident = cp.tile([128, 128], F32, name="ident")
            brow = cp.tile([2, 3 * HID], BF16, name="brow")
            ones2 = cp.tile([2, 128], BF16, name="ones2")
            b2rep = cp.tile([128, HID], F32, name="b2rep")
            b3rep = cp.tile([128, N3], F32, name="b3rep")
            wc2rep = cp.tile([128, HID], F32, name="wc2rep")
            fracpair = cp.tile([128, 2], F32, name="fracpair")
            lp0 = cp.tile([128, 2], F32, name="lp0")
            fpw = cp.tile([128, N_PASS * 2], F32, name="fpw")
            rmat = cp.tile([128, 128], F32, name="rmat")
            rmatw = cp.tile([128, N_PASS * 128], F32, name="rmatw")
            kmul = cp.tile([R, 3], F32, name="kmul")
            iota16 = cp.tile([R, 16], F32, name="iota16")
            nc.sync.dma_start(ident[:], ident_d.ap())
            nc.sync.dma_start(brow[:], brow_d.ap())
            nc.vector.memset(ones2[:], 1.0)
            nc.sync.dma_start(b2rep[:], b2rep_d.ap())
            nc.sync.dma_start(b3rep[:], b3rep_d.ap())
            nc.sync.dma_start(wc2rep[:], wc2rep_d.ap())
            nc.sync.dma_start(fracpair[:], fracpair_d.ap())
            nc.sync.dma_start(lp0[:], lp0_d.ap())
            nc.sync.dma_start(fpw[:].rearrange("p (n t) -> p n t", n=N_PASS),
                              fpw_d.ap().transpose([1, 0, 2]))
            nc.sync.dma_start(rmatw[:].rearrange("p (n q) -> p n q", n=N_PASS),
                              rmatw_d.ap().transpose([1, 0, 2]))
            nc.sync.dma_start(rmat[:], rmat_d.ap())
            nc.sync.dma_start(kmul[:], kmul_d.ap())
            nc.sync.dma_start(iota16[:], iota16_d.ap())
            consts = {"ident": ident, "fracpair": fracpair, "rmat": rmat,
                      "lp0": lp0, "fpw": fpw, "rmatw": rmatw,
                      "iota16": iota16}

            # ---- tail weights (once per launch, like the consts) ----
            w2sb = cp.tile([128, 8 * HID], F32, name="w2sb")
            w3sb = cp.tile([128, 4 * N3], F32R, name="w3sb")
            w4sb = cp.tile([128, 8 * N3], F32R, name="w4sb")
            nc.sync.dma_start(
                w2sb[:].rearrange("p (c w) -> p c w", c=8),
                w2t_d.ap().rearrange("(c p) w -> p c w", p=128))
            nc.sync.dma_start(
                w3sb[:].rearrange("p (c w) -> p c w", c=4),
                w3t_d.ap().rearrange("(c p) w -> p c w", p=128))
            nc.sync.dma_start(
                w4sb[:].rearrange("p (c w) -> p c w", c=8),
                w4t_d.ap().rearrange("(c p) w -> p c w", p=128))
            loop_ctx.__enter__()

            # ---- phase A: streamed big matmuls ----
            # one wide accumulator [bsz, 3*HID] per b_tile: each (pass, btile)
            # is a single matmul, sharing one stationary-weight load
            pacc = {}
            for bi, (bs, bsz) in enumerate(cfg.b_tiles):
                pacc[bi] = ap.tile([bsz, 3 * HID], F32, tag="acc", bufs=2,
                                   name=f"pacc{bi}")

            # PE p-state pre-warm: dummy matmuls on ident while the first
            # stream chunk is still in flight, so real matmuls start at the
            # max clock instead of spending their first ~3us ramping.
            if cfg.warmup:
                for wi in range(16):
                    warm = pst.tile([128, 128], F32, tag="tp", bufs=2,
                                    name=f"warm{wi}")
                    nc.tensor.matmul(warm[:], ident[:], ident[:, 0:128],
                                     start=True, stop=True)

            n_chunks = KT // chunk
            for cki in range(n_chunks):
                st = sp.tile([128, chunk * 2 * SW], BF16, tag="st", name=f"st{cki}")
                src = stream_d.ap()[chunk * cki: chunk * (cki + 1)]
                nc.sync.dma_start(
                    st[:].rearrange("p (c t w) -> p c t w", c=chunk, t=2),
                    src.transpose([1, 0, 2, 3]),
                )
                for ki in range(chunk):
                    kt = chunk * cki + ki
                    hi = st[:, (2 * ki) * SW:(2 * ki + 1) * SW]
                    lo = st[:, (2 * ki + 1) * SW:(2 * ki + 2) * SW]
                    first = kt == 0
                    # group the two hi-lhsT passes (hi*hi, hi*lo) under one
                    # stationary-weight load per b_tile; lo-lhsT pass last
                    for bi, (bs, bsz) in enumerate(cfg.b_tiles):
                        lhsT = hi[:, bs:bs + bsz]
                        for pi, wb in enumerate((hi, lo)):
                            f = first and pi == 0
                            for o in range(3):
                                nc.tensor.matmul(
                                    pacc[bi][:, HID * o:HID * (o + 1)], lhsT,
                                    wb[:, B + HID * o:B + HID * (o + 1)],
                                    start=f, stop=False)
                    for bi, (bs, bsz) in enumerate(cfg.b_tiles):
                        lhsT = lo[:, bs:bs + bsz]
                        for o in range(3):
                            nc.tensor.matmul(
                                pacc[bi][:, HID * o:HID * (o + 1)], lhsT,
                                hi[:, B + HID * o:B + HID * (o + 1)],
                                start=False, stop=False)
            # bias add via a K=2 matmul of ones against the hi/lo bias rows
            # (closes each accumulation group)
            for bi, (bs, bsz) in enumerate(cfg.b_tiles):
                for o in range(3):
                    nc.tensor.matmul(pacc[bi][:, HID * o:HID * (o + 1)],
                                     ones2[:, 0:bsz],
                                     brow[:, HID * o:HID * (o + 1)],
                                     start=False, stop=True)

            # ---- phase B: stage PSUM->SBUF (bias already added in-PE), RS ----
            rs_in = dram.tile([B, 3 * HID], F32, name="rs_in")
            rs_out = dram.tile([R, 3 * HID], F32, name="rs_out")
            for bi, (bs, bsz) in enumerate(cfg.b_tiles):
                so = sb.tile([bsz, 3 * HID], F32, tag="rsin_sb", bufs=2,
                             name=f"so{bi}")
                if bi == 0:
                    nc.vector.tensor_copy(so[:], pacc[bi][:])
                else:
                    nc.scalar.activation(so[:], pacc[bi][:], ACTF.Identity)
                nc.sync.dma_start(rs_in[bs:bs + bsz, :], so[:])
            if cfg.no_collective:
                nc.sync.dma_start(rs_out[:], rs_in[0:R, :])
            else:
                nc.gpsimd.collective_compute(
                    "ReduceScatter", ALU.add,
                    replica_groups=[list(range(NC))],
                    ins=[rs_in.opt()], outs=[rs_out.opt()],
                )

            # ---- phase C: replicated load + gate ----
            # replica 0 carries the full row (gate cols + x1); replicas 1-3
            # only need the x1 columns for the kwta probe replication.
            xall = sb.tile([128, 3 * HID], F32, name="xall")
            nc.sync.dma_start(xall[0:R, :], rs_out[:])
            for c in range(1, C):
                nc.sync.dma_start(xall[c * R:(c + 1) * R, HID:3 * HID],
                                  rs_out[:, HID:3 * HID])
            th = sb.tile([R, HID], F32, name="tanh")
            nc.scalar.activation(th[:], xall[0:R, 0:HID], ACTF.Tanh)
            ztr = sb.tile([R, HID], F32, name="ztr")
            z = sb.tile([R, 1], F32, name="z")
            nc.vector.scalar_tensor_tensor(
                ztr[:], th[:], 1.0, wc2rep[0:R, :], ALU.mult, ALU.mult,
                accum_out=z[:])
            ez = sb.tile([R, 1], F32, name="ez")
            nc.scalar.activation(ez[:], z[:], ACTF.Exp, scale=-1.0)
            ez1 = sb.tile([R, 1], F32, name="ez1")
            nc.vector.tensor_scalar(ez1[:], ez[:], 1.0, None, ALU.add)
            cx = sb.tile([R, 1], F32, name="cx")
            nc.vector.reciprocal(cx[:], ez1[:])
            kk = sb.tile([R, 3], F32, name="kk")
            nc.vector.tensor_scalar(kk[:], kmul[:], cx[:, 0:1], None, ALU.mult)
            kf3 = _floorize3(nc, sb, kk[:], R, "kf")
            # replicate the three k's to all 128 partitions via the PE
            krep_ps = pst.tile([128, 3], F32, tag="tp", bufs=2, name="krep_ps")
            nc.tensor.matmul(krep_ps[:], rmat[0:R, :], kf3[:], start=True, stop=True)
            krep = sb.tile([128, 3], F32, name="krep")
            nc.vector.tensor_copy(krep[:], krep_ps[:])
            
            do_tail = cfg.phase != "notail"
            if not do_tail:
                nt = sb.tile([R, N3], F32, name="nt")
                nc.vector.tensor_copy(nt[:], xall[0:R, HID:3 * HID])
                nc.vector.tensor_add(nt[:, 0:1], w2sb[0:R, 0:1], w2sb[0:R, 2:3])
                nc.vector.tensor_add(nt[:, 1:2], w2sb[0:R, 1:2], cx[0:R, :])
                nc.vector.tensor_add(nt[:, 2:3], krep[0:R, 0:1], krep[0:R, 1:2])
                nc.vector.tensor_add(nt[:, 3:4], krep[0:R, 2:3], fracpair[0:R, 1:2])
                nc.vector.tensor_add(nt[:, 4:5], ident[0:R, 0:1], iota16[0:R, 0:1])
                nc.vector.tensor_add(nt[:, 5:6], b2rep[0:R, 0:1], b3rep[0:R, 0:1])
                nc.sync.dma_start(out_d.ap(), nt[:])

            if do_tail:
                # ---- layer 1 kwta + mm2 ----
                x1 = xall[:, HID:3 * HID]
                masked1, thr1 = _kwta(nc, sb, pst, x1, krep, 0, N1, consts, "L1", cfg.pe_reduce)
                xt1 = _transpose_chunks(nc, sb, pst, masked1, N1, ident, True, "L1")
                px2 = pacc[0][:, 0:HID]
                w2v = w2sb[:].rearrange("p (c w) -> p c w", c=8)
                for ch in range(8):
                    nc.tensor.matmul(px2, xt1[ch][:], w2v[:, ch, :],
                                     start=(ch == 0), stop=(ch == 7))
                x2 = sb.tile([128, HID], F32, name="x2")
                nc.vector.tensor_add(x2[:], px2, b2rep[:])

                # ---- layer 2 kwta + mm3 ----
                masked2, thr2 = _kwta(nc, sb, pst, x2[:], krep, 1, HID, consts, "L2", cfg.pe_reduce)
                xt2 = _transpose_chunks(nc, sb, pst, masked2, HID, ident, True, "L2",
                                        dtype=F32R)
                px3 = pacc[0][:, HID:HID + N3]
                w3v = w3sb[:].rearrange("p (c w) -> p c w", c=4)
                for ch in range(4):
                    for o in range(2):
                        nc.tensor.matmul(
                            pacc[0][:, HID + 512 * o:HID + 512 * (o + 1)],
                            xt2[ch][:], w3v[:, ch, 512 * o:512 * (o + 1)],
                            start=(ch == 0), stop=(ch == 3))
                x3 = sb.tile([128, N3], F32, name="x3")
                nc.vector.tensor_add(x3[:], px3, b3rep[:])

                # ---- layer 3 kwta + mm4 ----
                masked3, thr3 = _kwta(nc, sb, pst, x3[:], krep, 2, N3, consts, "L3", cfg.pe_reduce)
                xt3 = _transpose_chunks(nc, sb, pst, masked3, N3, ident, False, "L3", dtype=F32R)
                px4 = pacc[1][0:R, 0:N3]
                w4v = w4sb[:].rearrange("p (c w) -> p c w", c=8)
                for ch in range(8):
                    for o in range(2):
                        nc.tensor.matmul(
                            pacc[1][0:R, 512 * o:512 * (o + 1)], xt3[ch][:],
                            w4v[:, ch, 512 * o:512 * (o + 1)],
                            start=(ch == 0), stop=(ch == 7))
                outsb = sb.tile([R, N3], F32, name="outsb")
                nc.vector.tensor_copy(outsb[:, 0:512], pacc[1][0:R, 0:512])
                nc.scalar.activation(outsb[:, 512:1024], pacc[1][0:R, 512:1024],
                                     ACTF.Identity)
                nc.sync.dma_start(out_d.ap(), outsb[:])
                if cfg.debug:
                    nc.sync.dma_start(dbg_rs_d.ap(), xall[0:R, :])
                    gsb = sb.tile([R, 8], F32, name="gsb")
                    nc.vector.tensor_copy(gsb[:, 0:1], cx[:])
                    nc.vector.tensor_copy(gsb[:, 1:2], krep[0:R, 0:1])
                    nc.vector.tensor_copy(gsb[:, 2:3], krep[0:R, 1:2])
                    nc.vector.tensor_copy(gsb[:, 3:4], krep[0:R, 2:3])
                    nc.vector.tensor_copy(gsb[:, 4:5], thr1[:])
                    nc.vector.tensor_copy(gsb[:, 5:6], thr2[:])
                    nc.vector.tensor_copy(gsb[:, 6:7], thr3[:])
                    nc.vector.tensor_copy(gsb[:, 7:8], z[:])
                    nc.sync.dma_start(dbg_gate_d.ap(), gsb[:])
                    nc.sync.dma_start(dbg_x2_d.ap(), x2[0:R, :])
                    nc.sync.dma_start(dbg_x3_d.ap(), x3[0:R, :])

            loop_ctx.__exit__(None, None, None)

    nc.compile()
    return nc


def host_prepare(inputs, cfg: Cfg):
    """Build per-core in_maps from the full inputs."""
    B, NC, KT, SW, KSH = cfg.B, cfg.NC, cfg.KT, cfg.SW, cfg.KSH
    f32 = np.float32
    inp = np.asarray(inputs["input"], f32)
    W_c1 = np.asarray(inputs["W_c1"], f32)
    b_c1 = np.asarray(inputs["b_c1"], f32)
    W_c2 = np.asarray(inputs["W_c2"], f32)
    W1 = np.asarray(inputs["W1"], f32)
    b1 = np.asarray(inputs["b1"], f32)
    W2 = np.asarray(inputs["W2"], f32)
    b2 = np.asarray(inputs["b2"], f32)
    W3 = np.asarray(inputs["W3"], f32)
    b3 = np.asarray(inputs["b3"], f32)
    W4 = np.asarray(inputs["W4"], f32)

    xT = np.ascontiguousarray(inp.T)          # [S2, B]
    wc1T = np.ascontiguousarray(W_c1.T)       # [S2, HID]
    w1T = np.ascontiguousarray(W1.T)          # [S2, N1]

    import ml_dtypes as _mld
    bcat = np.concatenate([b_c1, b1]).astype(f32) / NC
    bhi = bcat.astype(_mld.bfloat16)
    blo = (bcat - bhi.astype(f32)).astype(_mld.bfloat16)
    frac_col = ((np.arange(128, dtype=f32) // R + 1.0) / 5.0).astype(f32)
    pp = np.arange(128)
    consts = {
        "ident": np.eye(128, dtype=f32),
        "brow": np.stack([bhi, blo], axis=0),
        "b2rep": np.broadcast_to(b2, (128, HID)).copy(),
        "b3rep": np.broadcast_to(b3, (128, N3)).copy(),
        "wc2rep": np.broadcast_to(W_c2[0], (128, HID)).copy(),
        "fracpair": np.stack([np.zeros(128, f32), frac_col], axis=1).copy(),
        "lp0": np.stack([np.full(128, -16.0, f32),
                         (frac_col * 32.0 - 16.0).astype(f32)], axis=1).copy(),
        "fpw": np.stack([np.stack([np.zeros(128, f32),
                                   (frac_col * (32.0 * 0.2 ** (p + 1))).astype(f32)],
                                  axis=1)
                         for p in range(7)], axis=0).copy(),
        "rmat": (pp[:, None] % R == pp[None, :] % R).astype(f32),
        "rmatw": np.stack([(pp[:, None] % R == pp[None, :] % R).astype(f32)
                           * np.float32(32.0 * 0.2 ** (p + 1))
                           for p in range(7)], axis=0).copy(),
        "kmul": np.broadcast_to(
            np.array([2 * HID, HID, N3], f32), (R, 3)).copy(),
        "iota16": np.broadcast_to(np.arange(16, dtype=f32), (R, 16)).copy(),
        "w2t": np.ascontiguousarray(W2.T),
        "w3t": np.ascontiguousarray(W3.T),
        "w4t": np.ascontiguousarray(W4.T),
    }

    import ml_dtypes
    bf16 = ml_dtypes.bfloat16
    in_maps = []
    for c in range(NC):
        sl = slice(c * KSH, (c + 1) * KSH)
        stream = np.concatenate([xT[sl], wc1T[sl], w1T[sl]], axis=1)  # [KSH, SW]
        hi = stream.astype(bf16)
        lo = (stream - hi.astype(f32)).astype(bf16)
        shl = np.stack([hi, lo], axis=1).reshape(KSH, 2, cfg.SW)  # [KSH,2,SW]
        shl = np.ascontiguousarray(shl.reshape(KT, 128, 2, cfg.SW))
        in_maps.append({"stream": shl, **consts})
    return in_maps


_CACHE = {}


def kernel(**inputs) -> np.ndarray:
    cfg = Cfg(S2=inputs["input"].shape[1], B=inputs["input"].shape[0])
    key = (cfg.S2, cfg.B, cfg.NC)
    if key not in _CACHE:
        _CACHE[key] = build_nc(cfg)
    nc = _CACHE[key]
    in_maps = host_prepare(inputs, cfg)
    res = bass_utils.run_bass_kernel_spmd(
        nc, in_maps, core_ids=list(range(cfg.NC)))
    return np.concatenate([res.results[c]["out"] for c in range(cfg.NC)], axis=0)


if __name__ == "__main__":
    rng = np.random.default_rng(0)
    S2, B = 32768, 256
    demo = {
        "input": rng.standard_normal((B, S2), dtype=np.float32),
        "W_c1": rng.standard_normal((HID, S2), dtype=np.float32) / np.sqrt(S2),
        "b_c1": rng.standard_normal(HID).astype(np.float32) / np.sqrt(S2),
        "W_c2": rng.standard_normal((1, HID), dtype=np.float32) / np.sqrt(HID),
        "W1": rng.standard_normal((N1, S2), dtype=np.float32) / np.sqrt(S2),
        "b1": rng.standard_normal(N1).astype(np.float32) / np.sqrt(S2),
        "W2": rng.standard_normal((HID, N1), dtype=np.float32) / np.sqrt(N1),
        "b2": rng.standard_normal(HID).astype(np.float32) / np.sqrt(N1),
        "W3": rng.standard_normal((N3, HID), dtype=np.float32) / np.sqrt(HID),
        "b3": rng.standard_normal(N3).astype(np.float32) / np.sqrt(HID),
        "W4": rng.standard_normal((N3, N3), dtype=np.float32) / np.sqrt(N3),
    }
    out = kernel(**demo)
    print(out.shape, out.dtype, np.abs(out).max())

